# revision 46
# baseline (speedup 1.0000x reference)
"""Gated Linear Attention forward on 8 Trainium2 NeuronCores (Bass/Tile).

Problem: B=4, T=1024, D=1024, H=8, DK=64, DV=128, conv4 on q/k/v, low-rank
log-sigmoid forget gate, recurrent scan, RMS-norm + swish output gate, out proj.

Sharding: core = 2*b + hg  (b = batch, hg = half of the heads).
Each core computes its batch's tokens for 4 heads end-to-end and a partial
output projection (Wo row-block); the host sums the two partials per batch.

On-device algorithm: chunked-parallel GLA with chunk C=128.
Per chunk (local inclusive cumsum b of the log-gates):
  q~ = q * exp(b)/8,  k~ = k * exp(-b),  k^ = k~ * exp(b_C)
  A~[s,t] = sum_kk k~[s] q~[t]   masked to s<=t
  o = A~^T v (intra) + q~ @ S (inter), accumulated in one PSUM tile
  S' = diag(exp(b_C)) S + k^T v
Layouts: projections are computed transposed (channels on partitions, time on
free) so the depthwise conv is a per-partition-scalar shifted multiply-add;
v / k^ / gated-o are PE-transposed per 128x128 tile where time must sit on
partitions. Matmuls run as float32r (fp22 multiplies, fp32 accumulation); the
producers of every matmul operand write with f32r rounding to satisfy the BIR
verifier. q~/k~ are formed in place over the full T once the cumsum is ready.
The RMS rsqrt is deferred to a single Ln+Exp pass after the chunk loop so the
ACT engine never thrashes activation-table loads inside the loop.
"""

import numpy as np

import concourse.bass as bass
import concourse.mybir as mybir
import concourse.tile as tile
from concourse import bacc
from concourse.bass_utils import run_bass_kernel_spmd

F32 = mybir.dt.float32
F32R = mybir.dt.float32r
AF = mybir.ActivationFunctionType
OP = mybir.AluOpType

# problem constants (hardcoded per the task contract)
B, T, D, H = 4, 1024, 1024, 8
KD, VD = 512, 1024
DK, DV = 64, 128
CONV = 4
GATE_NORM = 16.0
EPS = 1e-5
LN8 = float(np.log(8.0))

# per-core shapes
KDC, VDC = 256, 512          # q/k and v/gate channels per core
MIQ, MIV = 2, 4              # 128-wide channel tiles for q/k and v
C, NCH = 128, 8              # chunk length, number of chunks
G = 2                        # head groups of 2 heads (128 chans) per core
NCORES = 8


def build_program():
    nc = bacc.Bacc("TRN2", target_bir_lowering=False, debug=False)

    # ---- DRAM I/O ----------------------------------------------------------
    srcT_d = nc.dram_tensor("srcT_in", [D, T], F32R, kind="ExternalInput")
    wq_d = nc.dram_tensor("wq", [D, KDC], F32R, kind="ExternalInput")
    wk_d = nc.dram_tensor("wk", [D, KDC], F32R, kind="ExternalInput")
    wv_d = nc.dram_tensor("wv", [D, VDC], F32R, kind="ExternalInput")
    wgate_d = nc.dram_tensor("wgate", [D, VDC], F32R, kind="ExternalInput")
    wg1_d = nc.dram_tensor("wg1", [D, 16], F32R, kind="ExternalInput")
    wg2b_d = nc.dram_tensor("wg2b", [17, KDC], F32R, kind="ExternalInput")
    wo_d = nc.dram_tensor("wo", [VDC, D], F32R, kind="ExternalInput")
    convq_d = nc.dram_tensor("convq", [128, MIQ * CONV], F32, kind="ExternalInput")
    convk_d = nc.dram_tensor("convk", [128, MIQ * CONV], F32, kind="ExternalInput")
    convv_d = nc.dram_tensor("convv", [128, MIV * CONV], F32, kind="ExternalInput")
    maskc_d = nc.dram_tensor("maskc", [128, NCH], F32, kind="ExternalInput")
    out_d = nc.dram_tensor("out", [T, D], F32, kind="ExternalOutput")

    ident_np = np.eye(128, dtype=np.float32)
    u = np.triu(np.ones((128, 128), np.float32))  # U[s,t] = 1 iff s <= t
    ident_d = nc.inline_tensor(ident_np, "ident_c")
    triu2_d = nc.inline_tensor(np.concatenate([u, u], axis=1), "triu2_c")

    # ---- static SBUF -------------------------------------------------------
    srcT = nc.alloc_sbuf_tensor("srcT", [128, 8, T], F32R)      # src^T, d-major
    q_sb = nc.alloc_sbuf_tensor("q_sb", [128, MIQ, T], F32R)     # q then q~ (in place)
    k_sb = nc.alloc_sbuf_tensor("k_sb", [128, MIQ, T], F32R)     # k then k~
    v_sb = nc.alloc_sbuf_tensor("v_sb", [128, MIV, T], F32)
    gate_sb = nc.alloc_sbuf_tensor("gate_sb", [128, NCH, VDC], F32)  # silu(gate) → o*gate
    xgT = nc.alloc_sbuf_tensor("xgT", [17, T], F32R)            # (src@Wg1)^T + ones row
    spT = nc.alloc_sbuf_tensor("spT", [128, MIQ, T], F32)       # softplus(-gk_logit)
    bsum = nc.alloc_sbuf_tensor("bsum", [128, MIQ, T], F32)     # per-chunk cumsum of spT
    bCn = nc.alloc_sbuf_tensor("bCn", [128, MIQ, NCH], F32)     # -spsum_last/16 per chunk
    ssq_all = nc.alloc_sbuf_tensor("ssq_all", [128, NCH * 4], F32)   # col = c*4 + head
    rrms_all = nc.alloc_sbuf_tensor("rrms_all", [128, NCH * 4], F32)
    Eall = nc.alloc_sbuf_tensor("Eall", [128, MIQ, NCH], F32)   # exp(b_C) per chunk
    wo_sb = nc.alloc_sbuf_tensor("wo_sb", [128, MIV, D], F32R)
    wgate_sb = nc.alloc_sbuf_tensor("wgate_sb", [128, 8, VDC], F32R)
    wg1_sb = nc.alloc_sbuf_tensor("wg1_sb", [128, 8, 16], F32R)
    wg2b_sb = nc.alloc_sbuf_tensor("wg2b_sb", [17, KDC], F32R)
    convq_sb = nc.alloc_sbuf_tensor("convq_sb", [128, MIQ * CONV], F32)
    convk_sb = nc.alloc_sbuf_tensor("convk_sb", [128, MIQ * CONV], F32)
    convv_sb = nc.alloc_sbuf_tensor("convv_sb", [128, MIV * CONV], F32)
    maskc_sb = nc.alloc_sbuf_tensor("maskc_sb", [128, NCH], F32)
    ident = nc.alloc_sbuf_tensor("ident", [128, 128], F32)
    triu2 = nc.alloc_sbuf_tensor("triu2", [128, 256], F32)
    ones_sb = nc.alloc_sbuf_tensor("ones_sb", [128, 128], F32)
    Sblk = [nc.alloc_sbuf_tensor(f"Sblk{g}", [128, 256], F32R) for g in range(G)]
    qblk = [nc.alloc_sbuf_tensor(f"qblk{g}", [128, 256], F32R) for g in range(G)]
    negln8 = nc.alloc_sbuf_tensor("negln8", [128, 1], F32)
    eps_col = nc.alloc_sbuf_tensor("eps_col", [128, 1], F32)

    with tile.TileContext(nc) as tc:
        with (
            tc.tile_pool(name="scr", bufs=4) as scr,
            tc.tile_pool(name="ps_t", bufs=2, space="PSUM") as ps_t,
        ):
            # ---- phase 0: constants in (src streams first; the big late
            # weights go on the GpSimd DMA queue so they don't delay src) ----
            nc.sync.dma_start(out=ident[:], in_=ident_d[:])
            nc.sync.dma_start(out=triu2[:], in_=triu2_d[:])
            nc.sync.dma_start(out=maskc_sb[:], in_=maskc_d[:])
            nc.sync.dma_start(out=convq_sb[:], in_=convq_d[:])
            nc.sync.dma_start(out=convk_sb[:], in_=convk_d[:])
            nc.sync.dma_start(out=convv_sb[:], in_=convv_d[:])
            nc.sync.dma_start(
                out=wg1_sb[:], in_=wg1_d[:].rearrange("(kt p) m -> p kt m", p=128)
            )
            nc.sync.dma_start(out=wg2b_sb[:], in_=wg2b_d[:])
            nc.vector.memset(ones_sb[:], 1.0)
            # row 16 must be ones (bias row); rows 0..15 are overwritten later
            nc.vector.memset(xgT[:].bitcast(F32), 1.0)
            nc.vector.memset(negln8[:], -LN8)
            nc.vector.memset(eps_col[:], EPS)
            for g in range(G):
                nc.vector.memset(Sblk[g][:].bitcast(F32), 0.0)
                nc.vector.memset(qblk[g][:].bitcast(F32), 0.0)

            # ---- phases 1+2: gk; projections+conv; gate --------------------
            # (src arrives pre-transposed from the host: one straight DMA)
            with (
                tc.tile_pool(name="wmi", bufs=2) as wmi_pool,
                tc.tile_pool(name="scr2", bufs=2) as scr2,
                tc.tile_pool(name="ps_proj", bufs=6, space="PSUM") as ps_proj,
            ):
                # eight separate DMAs so the 4MB spreads across DMA queues
                for kt in range(8):
                    nc.sync.dma_start(
                        out=srcT[:, kt, :],
                        in_=srcT_d[kt * 128:(kt + 1) * 128, :],
                    )

                # gk path first so the chunk loop's inputs are ready early:
                # xg^T = (src @ Wg1)^T with an appended ones row
                for nh in range(2):
                    p = ps_proj.tile([128, 512], F32, name="pp_xg", tag="pp")
                    for kt in range(8):
                        nc.tensor.matmul(
                            p[0:16, :],
                            wg1_sb[:, kt, :],
                            srcT[:, kt, nh * 512:(nh + 1) * 512],
                            start=(kt == 0),
                            stop=(kt == 7),
                        )
                    nc.vector.tensor_copy(
                        out=xgT[0:16, nh * 512:(nh + 1) * 512], in_=p[0:16, :]
                    )
                # spT = softplus(-(xg @ Wg2 + bg2)) = log(1 + exp(-logit))
                # (logits here are O(1), so exp(-x) cannot overflow).
                # All 4 Exps run before all 4 Lns so the ACT engine loads
                # each activation table once instead of thrashing.
                enxs = []
                for mi in range(MIQ):
                    for nh in range(2):
                        p = ps_proj.tile([128, 512], F32, name="pp_sp", tag="pp")
                        nc.tensor.matmul(
                            p[:],
                            wg2b_sb[:, mi * 128:(mi + 1) * 128],
                            xgT[:, nh * 512:(nh + 1) * 512],
                            start=True,
                            stop=True,
                        )
                        enx = scr2.tile(
                            [128, 512], F32, name="enx", tag="enx", bufs=4
                        )
                        nc.scalar.activation(enx[:], p[:], AF.Exp, scale=-1.0)
                        enxs.append((mi, nh, enx))
                for mi, nh, enx in enxs:
                    nc.scalar.activation(
                        spT[:, mi, nh * 512:(nh + 1) * 512], enx[:],
                        AF.Ln, bias=1.0,
                    )
                # per-chunk inclusive cumsum of spT, chunk-end columns
                for mi in range(MIQ):
                    for c in range(NCH):
                        csl = slice(c * 128, (c + 1) * 128)
                        nc.vector.tensor_tensor_scan(
                            out=bsum[:, mi, csl],
                            data0=ones_sb[:],
                            data1=spT[:, mi, csl],
                            initial=0.0,
                            op0=OP.mult,
                            op1=OP.add,
                        )
                        nc.vector.tensor_scalar_mul(
                            bCn[:, mi, c:c + 1],
                            bsum[:, mi, c * 128 + 127:c * 128 + 128],
                            -1.0 / GATE_NORM,
                        )
                    nc.scalar.activation(Eall[:, mi, :], bCn[:, mi, :], AF.Exp)

                def conv_proj(w_dram, conv_sb, dst, mi_count, prio_tag):
                    """dst[:, mi, :] = silu(conv4(src @ W[:, mi-block]))^T.

                    The pre-activation is staged (padded) in SBUF via ACT
                    copies, so each conv tap is one full-span shifted
                    multiply-add; alternate tiles run on DVE vs GpSimd.
                    """
                    for mi in range(mi_count):
                        w_mi = wmi_pool.tile(
                            [128, 8, 128], F32R, name="w_mi", tag="w_mi"
                        )
                        nc.scalar.dma_start(
                            out=w_mi[:],
                            in_=w_dram[:, mi * 128:(mi + 1) * 128].rearrange(
                                "(kt p) m -> p kt m", p=128
                            ),
                        )
                        pre = scr2.tile([128, 1027], F32, name="pre", tag="pre")
                        nc.gpsimd.memset(pre[:, 0:3], 0.0)
                        for nh in range(2):
                            p = ps_proj.tile(
                                [128, 512], F32, name=f"pp_{prio_tag}", tag="pp"
                            )
                            for kt in range(8):
                                nc.tensor.matmul(
                                    p[:],
                                    w_mi[:, kt, :],
                                    srcT[:, kt, nh * 512:(nh + 1) * 512],
                                    start=(kt == 0),
                                    stop=(kt == 7),
                                )
                            nc.scalar.copy(
                                out=pre[:, 3 + nh * 512:3 + (nh + 1) * 512], in_=p[:]
                            )
                        # causal conv: out[t] = sum_j pre_padded[t + j] w[:, j]
                        # (GpSimd has no TensorScalarPtr, so taps stay on DVE)
                        dst_mi = dst[:, mi, :]
                        w_of = lambda j: conv_sb[:, mi * CONV + j: mi * CONV + j + 1]
                        nc.vector.tensor_scalar_mul(dst_mi[:], pre[:, 3:1027], w_of(3))
                        for j in range(2, -1, -1):
                            nc.vector.scalar_tensor_tensor(
                                out=dst_mi[:],
                                in0=pre[:, j:j + 1024],
                                scalar=w_of(j),
                                in1=dst_mi[:],
                                op0=OP.mult,
                                op1=OP.add,
                            )
                        # silu(x) = x * sigmoid(x); multiply on idle GpSimd
                        sg = scr2.tile([128, 1024], F32, name="sg", tag="sg")
                        nc.scalar.activation(sg[:], dst_mi[:], AF.Sigmoid)
                        nc.gpsimd.tensor_mul(dst_mi[:], dst_mi[:], sg[:])

                nc.gpsimd.dma_start(
                    out=wgate_sb[:],
                    in_=wgate_d[:].rearrange("(kt p) m -> p kt m", p=128),
                )
                conv_proj(wq_d[:], convq_sb, q_sb, MIQ, "q")
                conv_proj(wk_d[:], convk_sb, k_sb, MIQ, "k")

                # q~ = q exp(b)/8 and k~ = k exp(-b), full-T, in place, with
                # f32r rounding on the write (they feed matmuls from here on).
                # Emitted right after the q/k convs so the chunk loop can start
                # while the v conv and gate projection are still running.
                for mi in range(MIQ):
                    texp = scr2.tile([128, 1024], F32, name="texp", tag="texp")
                    nc.scalar.activation(
                        texp[:], bsum[:, mi, :], AF.Exp,
                        scale=-1.0 / GATE_NORM, bias=negln8[:],
                    )
                    nc.gpsimd.tensor_mul(
                        q_sb[:, mi, :], q_sb[:, mi, :], texp[:]
                    )
                    texp2 = scr2.tile([128, 1024], F32, name="texp2", tag="texp")
                    nc.scalar.activation(
                        texp2[:], bsum[:, mi, :], AF.Exp, scale=1.0 / GATE_NORM,
                    )
                    nc.gpsimd.tensor_mul(
                        k_sb[:, mi, :], k_sb[:, mi, :], texp2[:]
                    )

                # gate: silu(src @ Wgate), natural (t-major) layout — dense PE
                # work that overlaps the conv's DVE stretch
                for mt in range(8):
                    p = ps_proj.tile([128, 512], F32, name="pp_gate", tag="pp")
                    for kt in range(8):
                        nc.tensor.matmul(
                            p[:],
                            srcT[:, kt, mt * 128:(mt + 1) * 128],
                            wgate_sb[:, kt, :],
                            start=(kt == 0),
                            stop=(kt == 7),
                        )
                    sgg = scr2.tile([128, 512], F32, name="sgg", tag="sgg")
                    nc.scalar.activation(sgg[:], p[:], AF.Sigmoid)
                    nc.vector.tensor_mul(gate_sb[:, mt, :], p[:], sgg[:])

                nc.gpsimd.dma_start(
                    out=wo_sb[:], in_=wo_d[:].rearrange("(h p) m -> p h m", p=128)
                )
                conv_proj(wv_d[:], convv_sb, v_sb, MIV, "v")

            # ---- phase 4: GLA chunk recurrence -----------------------------
            with (
                tc.tile_pool(name="ogT_pool", bufs=1) as ogT_pool,
            ):
                ogT = ogT_pool.tile([128, MIV, T], F32R, name="ogT")
                with (
                    tc.tile_pool(name="ps_h", bufs=4, space="PSUM") as ps_h,
                    tc.tile_pool(name="ps_o", bufs=2, space="PSUM") as ps_o_pool,
                ):
                    for c in range(NCH):
                        csl = slice(c * 128, (c + 1) * 128)
                        for g in range(G):
                            qt = q_sb[:, g, csl]
                            kt_ = k_sb[:, g, csl]
                            e_col = Eall[:, g, c:c + 1]
                            # k^ = k~ * exp(b_C)  (per-partition scalar)
                            kh_s = scr.tile([128, 128], F32, name="kh_s", tag="kh_s")
                            nc.vector.tensor_scalar_mul(kh_s[:], k_sb[:, g, csl], e_col)
                            nc.vector.tensor_copy(
                                out=qblk[g][0:64, 0:128], in_=qt[0:64, :]
                            )
                            nc.vector.tensor_copy(
                                out=qblk[g][64:128, 128:256], in_=qt[64:128, :]
                            )
                            # A~[s, t] for both heads: (s, [t_h0 | t_h1])
                            ps_a = ps_h.tile([128, 256], F32, name="ps_a", tag="ps_h")
                            nc.tensor.matmul(
                                ps_a[:], kt_, qblk[g][:], start=True, stop=True
                            )
                            a_sb = scr.tile([128, 256], F32R, name="a_sb", tag="a_sb")
                            nc.vector.tensor_mul(a_sb[:], ps_a[:], triu2[:])
                            # v chunk, time-major (+ padding mask)
                            ps_v = ps_h.tile([128, 256], F32, name="ps_v", tag="ps_h")
                            nc.tensor.matmul(
                                ps_v[:, 0:128], v_sb[:, 2 * g, csl], ident[:],
                                is_transpose=True, start=True, stop=False,
                                skip_group_check=True,
                            )
                            nc.tensor.matmul(
                                ps_v[:, 128:256], v_sb[:, 2 * g + 1, csl], ident[:],
                                is_transpose=True, start=False, stop=True,
                                skip_group_check=True,
                            )
                            vnat = scr.tile([128, 256], F32R, name="vnat", tag="vnat")
                            nc.vector.tensor_scalar_mul(
                                vnat[:], ps_v[:], maskc_sb[:, c:c + 1]
                            )
                            # k^ chunk, time-major
                            ps_k = ps_t.tile([128, 256], F32, name="ps_k", tag="pst")
                            nc.tensor.transpose(ps_k[:, 0:128], kh_s[:], ident[:])
                            khnat = scr.tile([128, 128], F32R, name="khnat", tag="khnat")
                            nc.scalar.copy(out=khnat[:], in_=ps_k[:, 0:128])
                            # o = A~^T v (intra) + q~ @ S (inter)
                            ps_o = ps_o_pool.tile([128, 256], F32, name="ps_o", tag="ps_o")
                            nc.tensor.matmul(
                                ps_o[:, 0:128], a_sb[:, 0:128], vnat[:, 0:128],
                                start=True, stop=False, skip_group_check=True,
                            )
                            nc.tensor.matmul(
                                ps_o[:, 128:256], a_sb[:, 128:256], vnat[:, 128:256],
                                start=False, stop=False, skip_group_check=True,
                            )
                            nc.tensor.matmul(
                                ps_o[:], qt, Sblk[g][:],
                                start=False, stop=True, skip_group_check=True,
                            )
                            # state update: S = diag(exp(b_C)) S + k^T v
                            ps_s = ps_h.tile([128, 256], F32, name="ps_s", tag="ps_h")
                            nc.tensor.matmul(
                                ps_s[:], khnat[:], vnat[:], start=True, stop=True
                            )
                            nc.vector.scalar_tensor_tensor(
                                out=Sblk[g][0:64, 0:128],
                                in0=Sblk[g][0:64, 0:128],
                                scalar=e_col[0:64, :],
                                in1=ps_s[0:64, 0:128],
                                op0=OP.mult,
                                op1=OP.add,
                            )
                            nc.vector.scalar_tensor_tensor(
                                out=Sblk[g][64:128, 128:256],
                                in0=Sblk[g][64:128, 128:256],
                                scalar=e_col[64:128, :],
                                in1=ps_s[64:128, 128:256],
                                op0=OP.mult,
                                op1=OP.add,
                            )
                            # evacuate o: multiply the swish gate in-place into
                            # gate_sb, and collect per-head sums of squares
                            for lh in range(2):
                                sqd = scr.tile([128, 128], F32, name="sqd", tag="sqd")
                                idx = c * 4 + 2 * g + lh
                                nc.scalar.activation(
                                    sqd[:], ps_o[:, lh * 128:(lh + 1) * 128],
                                    AF.Square,
                                    accum_out=ssq_all[:, idx:idx + 1],
                                )
                            gsl = slice(g * 256, (g + 1) * 256)
                            nc.vector.tensor_mul(
                                gate_sb[:, c, gsl], ps_o[:], gate_sb[:, c, gsl]
                            )

                # ---- tail: rrms, gate-scale, transpose, and the output
                # projection all pipelined per chunk (chunk c is exactly
                # output row-tile mt=c, so each chunk streams straight
                # through Wo and out to DRAM)
                lnr = scr.tile([128, NCH * 4], F32, name="lnr", tag="lnr")
                nc.scalar.activation(
                    lnr[:], ssq_all[:], AF.Ln, scale=1.0 / DV, bias=eps_col[:]
                )
                nc.scalar.activation(rrms_all[:], lnr[:], AF.Exp, scale=-0.5)
                with (
                    tc.tile_pool(name="ps_out", bufs=3, space="PSUM") as ps_out,
                    tc.tile_pool(name="stage", bufs=3) as stage_pool,
                ):
                    for c in range(NCH):
                        csl = slice(c * 128, (c + 1) * 128)
                        rr = rrms_all[:, c * 4:(c + 1) * 4, None].to_broadcast(
                            (128, 4, 128)
                        )
                        nc.gpsimd.tensor_mul(
                            gate_sb[:, c, :].rearrange("p (h x) -> p h x", h=4),
                            gate_sb[:, c, :].rearrange("p (h x) -> p h x", h=4),
                            rr,
                        )
                        for h in range(0, 4, 2):
                            ps_g = ps_t.tile([128, 256], F32, name="ps_g", tag="pst")
                            nc.tensor.matmul(
                                ps_g[:, 0:128], gate_sb[:, c, h * 128:(h + 1) * 128],
                                ident[:], is_transpose=True, start=True, stop=False,
                                skip_group_check=True,
                            )
                            nc.tensor.matmul(
                                ps_g[:, 128:256],
                                gate_sb[:, c, (h + 1) * 128:(h + 2) * 128],
                                ident[:], is_transpose=True, start=False, stop=True,
                                skip_group_check=True,
                            )
                            nc.scalar.copy(
                                out=ogT[:, h:h + 2, csl],
                                in_=ps_g[:].rearrange("p (a b) -> p a b", a=2),
                            )
                        for nh in range(2):
                            p = ps_out.tile([128, 512], F32, name="p_out", tag="p_out")
                            for h in range(4):
                                nc.tensor.matmul(
                                    p[:],
                                    ogT[:, h, csl],
                                    wo_sb[:, h, nh * 512:(nh + 1) * 512],
                                    start=(h == 0),
                                    stop=(h == 3),
                                )
                            stage = stage_pool.tile(
                                [128, 512], F32, name="stage", tag="stage"
                            )
                            if (c + nh) % 2 == 0:
                                nc.vector.tensor_copy(out=stage[:], in_=p[:])
                            else:
                                nc.scalar.copy(out=stage[:], in_=p[:])
                            nc.gpsimd.dma_start(
                                out=out_d[c * 128:(c + 1) * 128,
                                          nh * 512:(nh + 1) * 512],
                                in_=stage[:],
                            )

    nc.compile()
    return nc


_NC_CACHE = None


def _get_program():
    global _NC_CACHE
    if _NC_CACHE is None:
        _NC_CACHE = build_program()
    return _NC_CACHE


def shard_inputs(
    src, valid_mask, Wq, Wk, Wv, conv_q_w, conv_k_w, conv_v_w,
    Wg1, Wg2, bg2, Wgate, rms_w, Wo,
):
    """Build the 8 per-core input maps."""
    f = np.float32
    src = np.asarray(src, f)
    valid_mask = np.asarray(valid_mask)
    in_maps = []
    wo_scaled = np.asarray(Wo, f) * np.tile(np.asarray(rms_w, f), VD // DV)[:, None]
    for core in range(NCORES):
        b, hg = core // 2, core % 2
        qs = slice(hg * KDC, (hg + 1) * KDC)
        vs = slice(hg * VDC, (hg + 1) * VDC)
        wg2b = np.concatenate(
            [np.asarray(Wg2, f)[:, qs], np.asarray(bg2, f)[None, qs]], axis=0
        )

        def conv_fold(w, mi_count):
            w = np.asarray(w, f)  # (chans, 4) slice for this core
            return np.ascontiguousarray(
                w.reshape(mi_count, 128, CONV).transpose(1, 0, 2).reshape(128, -1)
            )

        in_maps.append({
            "srcT_in": np.ascontiguousarray(src[b].T),
            "wq": np.ascontiguousarray(np.asarray(Wq, f)[:, qs]),
            "wk": np.ascontiguousarray(np.asarray(Wk, f)[:, qs]),
            "wv": np.ascontiguousarray(np.asarray(Wv, f)[:, vs]),
            "wgate": np.ascontiguousarray(np.asarray(Wgate, f)[:, vs]),
            "wg1": np.ascontiguousarray(np.asarray(Wg1, f)),
            "wg2b": np.ascontiguousarray(wg2b),
            "wo": np.ascontiguousarray(wo_scaled[vs, :]),
            "convq": conv_fold(np.asarray(conv_q_w, f)[qs], MIQ),
            "convk": conv_fold(np.asarray(conv_k_w, f)[qs], MIQ),
            "convv": conv_fold(np.asarray(conv_v_w, f)[vs], MIV),
            "maskc": np.ascontiguousarray(
                valid_mask[b].astype(f).reshape(NCH, 128).T
            ),
        })
    return in_maps


def kernel(**inputs):
    nc = _get_program()
    in_maps = shard_inputs(**inputs)
    res = run_bass_kernel_spmd(nc, in_maps, list(range(NCORES)))
    out = np.zeros((B, T, D), np.float32)
    for core in range(NCORES):
        out[core // 2] += res.results[core]["out"]
    return out


if __name__ == "__main__":
    prog = _get_program()
    print("program built OK")


# revision 48
# speedup vs baseline: 1.0330x; 1.0330x over previous
"""Gated Linear Attention forward on 8 Trainium2 NeuronCores (Bass/Tile).

Problem: B=4, T=1024, D=1024, H=8, DK=64, DV=128, conv4 on q/k/v, low-rank
log-sigmoid forget gate, recurrent scan, RMS-norm + swish output gate, out proj.

Sharding: core = 2*b + hg  (b = batch, hg = half of the heads).
Each core computes its batch's tokens for 4 heads end-to-end and a partial
output projection (Wo row-block); the host sums the two partials per batch.

On-device algorithm: chunked-parallel GLA with chunk C=128.
Per chunk (local inclusive cumsum b of the log-gates):
  q~ = q * exp(b)/8,  k~ = k * exp(-b),  k^ = k~ * exp(b_C)
  A~[s,t] = sum_kk k~[s] q~[t]   masked to s<=t
  o = A~^T v (intra) + q~ @ S (inter), accumulated in one PSUM tile
  S' = diag(exp(b_C)) S + k^T v
Layouts: projections are computed transposed (channels on partitions, time on
free) so the depthwise conv is a per-partition-scalar shifted multiply-add;
v / k^ / gated-o are PE-transposed per 128x128 tile where time must sit on
partitions. Matmuls run as float32r (fp22 multiplies, fp32 accumulation); the
producers of every matmul operand write with f32r rounding to satisfy the BIR
verifier. q~/k~ are formed in place over the full T once the cumsum is ready.
The RMS rsqrt is deferred to a single Ln+Exp pass after the chunk loop so the
ACT engine never thrashes activation-table loads inside the loop.
"""

import numpy as np

import concourse.bass as bass
import concourse.mybir as mybir
import concourse.tile as tile
from concourse import bacc
from concourse.bass_utils import run_bass_kernel_spmd

F32 = mybir.dt.float32
F32R = mybir.dt.float32r
AF = mybir.ActivationFunctionType
OP = mybir.AluOpType

# problem constants (hardcoded per the task contract)
B, T, D, H = 4, 1024, 1024, 8
KD, VD = 512, 1024
DK, DV = 64, 128
CONV = 4
GATE_NORM = 16.0
EPS = 1e-5
LN8 = float(np.log(8.0))

# per-core shapes
KDC, VDC = 256, 512          # q/k and v/gate channels per core
MIQ, MIV = 2, 4              # 128-wide channel tiles for q/k and v
C, NCH = 128, 8              # chunk length, number of chunks
G = 2                        # head groups of 2 heads (128 chans) per core
NCORES = 8


def build_program():
    nc = bacc.Bacc("TRN2", target_bir_lowering=False, debug=False)

    # ---- DRAM I/O ----------------------------------------------------------
    srcT_d = nc.dram_tensor("srcT_in", [D, T], F32R, kind="ExternalInput")
    wq_d = nc.dram_tensor("wq", [D, KDC], F32R, kind="ExternalInput")
    wk_d = nc.dram_tensor("wk", [D, KDC], F32R, kind="ExternalInput")
    wv_d = nc.dram_tensor("wv", [D, VDC], F32R, kind="ExternalInput")
    wgate_d = nc.dram_tensor("wgate", [D, VDC], F32R, kind="ExternalInput")
    wg1_d = nc.dram_tensor("wg1", [D, 16], F32R, kind="ExternalInput")
    wg2b_d = nc.dram_tensor("wg2b", [17, KDC], F32R, kind="ExternalInput")
    wo_d = nc.dram_tensor("wo", [VDC, D], F32R, kind="ExternalInput")
    convq_d = nc.dram_tensor("convq", [128, MIQ * CONV], F32, kind="ExternalInput")
    convk_d = nc.dram_tensor("convk", [128, MIQ * CONV], F32, kind="ExternalInput")
    convv_d = nc.dram_tensor("convv", [128, MIV * CONV], F32, kind="ExternalInput")
    maskc_d = nc.dram_tensor("maskc", [128, NCH], F32, kind="ExternalInput")
    out_d = nc.dram_tensor("out", [T, D], F32, kind="ExternalOutput")

    ident_np = np.eye(128, dtype=np.float32)
    u = np.triu(np.ones((128, 128), np.float32))  # U[s,t] = 1 iff s <= t
    ident_d = nc.inline_tensor(ident_np, "ident_c")
    triu2_d = nc.inline_tensor(np.concatenate([u, u], axis=1), "triu2_c")

    # ---- static SBUF -------------------------------------------------------
    srcT = nc.alloc_sbuf_tensor("srcT", [128, 8, T], F32R)      # src^T, d-major
    q_sb = nc.alloc_sbuf_tensor("q_sb", [128, MIQ, T], F32R)     # q then q~ (in place)
    k_sb = nc.alloc_sbuf_tensor("k_sb", [128, MIQ, T], F32R)     # k then k~
    v_sb = nc.alloc_sbuf_tensor("v_sb", [128, MIV, T], F32)
    gate_sb = nc.alloc_sbuf_tensor("gate_sb", [128, NCH, VDC], F32)  # silu(gate) → o*gate
    xgT = nc.alloc_sbuf_tensor("xgT", [17, T], F32R)            # (src@Wg1)^T + ones row
    spT = nc.alloc_sbuf_tensor("spT", [128, MIQ, T], F32)       # softplus(-gk_logit)
    bsum = nc.alloc_sbuf_tensor("bsum", [128, MIQ, T], F32)     # per-chunk cumsum of spT
    bCn = nc.alloc_sbuf_tensor("bCn", [128, MIQ, NCH], F32)     # -spsum_last/16 per chunk
    ssq_all = nc.alloc_sbuf_tensor("ssq_all", [128, NCH * 4], F32)   # col = c*4 + head
    rrms_all = nc.alloc_sbuf_tensor("rrms_all", [128, NCH * 4], F32)
    Eall = nc.alloc_sbuf_tensor("Eall", [128, MIQ, NCH], F32)   # exp(b_C) per chunk
    wo_sb = nc.alloc_sbuf_tensor("wo_sb", [128, MIV, D], F32R)
    wgate_sb = nc.alloc_sbuf_tensor("wgate_sb", [128, 8, VDC], F32R)
    wg1_sb = nc.alloc_sbuf_tensor("wg1_sb", [128, 8, 16], F32R)
    wg2b_sb = nc.alloc_sbuf_tensor("wg2b_sb", [17, KDC], F32R)
    convq_sb = nc.alloc_sbuf_tensor("convq_sb", [128, MIQ * CONV], F32)
    convk_sb = nc.alloc_sbuf_tensor("convk_sb", [128, MIQ * CONV], F32)
    convv_sb = nc.alloc_sbuf_tensor("convv_sb", [128, MIV * CONV], F32)
    maskc_sb = nc.alloc_sbuf_tensor("maskc_sb", [128, NCH], F32)
    ident = nc.alloc_sbuf_tensor("ident", [128, 128], F32)
    triu2 = nc.alloc_sbuf_tensor("triu2", [128, 256], F32)
    ones_sb = nc.alloc_sbuf_tensor("ones_sb", [128, 128], F32)
    Sblk = [nc.alloc_sbuf_tensor(f"Sblk{g}", [128, 256], F32R) for g in range(G)]
    qblk = [nc.alloc_sbuf_tensor(f"qblk{g}", [128, 256], F32R) for g in range(G)]
    negln8 = nc.alloc_sbuf_tensor("negln8", [128, 1], F32)
    eps_col = nc.alloc_sbuf_tensor("eps_col", [128, 1], F32)

    with tile.TileContext(nc) as tc:
        with (
            tc.tile_pool(name="scr", bufs=4) as scr,
            tc.tile_pool(name="ps_t", bufs=2, space="PSUM") as ps_t,
        ):
            # ---- phase 0: constants in (src streams first; the big late
            # weights go on the GpSimd DMA queue so they don't delay src) ----
            nc.sync.dma_start(out=ident[:], in_=ident_d[:])
            nc.sync.dma_start(out=triu2[:], in_=triu2_d[:])
            nc.sync.dma_start(out=maskc_sb[:], in_=maskc_d[:])
            nc.sync.dma_start(out=convq_sb[:], in_=convq_d[:])
            nc.sync.dma_start(out=convk_sb[:], in_=convk_d[:])
            nc.sync.dma_start(out=convv_sb[:], in_=convv_d[:])
            nc.sync.dma_start(
                out=wg1_sb[:], in_=wg1_d[:].rearrange("(kt p) m -> p kt m", p=128)
            )
            nc.sync.dma_start(out=wg2b_sb[:], in_=wg2b_d[:])
            nc.vector.memset(ones_sb[:], 1.0)
            # row 16 must be ones (bias row); rows 0..15 are overwritten later
            nc.vector.memset(xgT[:].bitcast(F32), 1.0)
            nc.vector.memset(negln8[:], -LN8)
            nc.vector.memset(eps_col[:], EPS)
            for g in range(G):
                nc.vector.memset(Sblk[g][:].bitcast(F32), 0.0)
                nc.vector.memset(qblk[g][:].bitcast(F32), 0.0)

            # ---- phases 1+2: gk; projections+conv; gate --------------------
            # (src arrives pre-transposed from the host: one straight DMA)
            with (
                tc.tile_pool(name="wmi", bufs=4) as wmi_pool,
                tc.tile_pool(name="scr2", bufs=2) as scr2,
                tc.tile_pool(name="ps_proj", bufs=6, space="PSUM") as ps_proj,
            ):
                # eight separate DMAs, issued from all three DMA-capable
                # engines so the 4MB spreads across independent DMA paths
                dma_engs = [nc.sync, nc.scalar, nc.gpsimd]
                for kt in range(8):
                    dma_engs[kt % 3].dma_start(
                        out=srcT[:, kt, :],
                        in_=srcT_d[kt * 128:(kt + 1) * 128, :],
                    )

                # gk path first so the chunk loop's inputs are ready early:
                # xg^T = (src @ Wg1)^T with an appended ones row
                for nh in range(2):
                    p = ps_proj.tile([128, 512], F32, name="pp_xg", tag="pp")
                    for kt in range(8):
                        nc.tensor.matmul(
                            p[0:16, :],
                            wg1_sb[:, kt, :],
                            srcT[:, kt, nh * 512:(nh + 1) * 512],
                            start=(kt == 0),
                            stop=(kt == 7),
                        )
                    nc.vector.tensor_copy(
                        out=xgT[0:16, nh * 512:(nh + 1) * 512], in_=p[0:16, :]
                    )
                # spT = softplus(-(xg @ Wg2 + bg2)) = log(1 + exp(-logit))
                # (logits here are O(1), so exp(-x) cannot overflow).
                # All 4 Exps run before all 4 Lns so the ACT engine loads
                # each activation table once instead of thrashing.
                enxs = []
                for mi in range(MIQ):
                    for nh in range(2):
                        p = ps_proj.tile([128, 512], F32, name="pp_sp", tag="pp")
                        nc.tensor.matmul(
                            p[:],
                            wg2b_sb[:, mi * 128:(mi + 1) * 128],
                            xgT[:, nh * 512:(nh + 1) * 512],
                            start=True,
                            stop=True,
                        )
                        enx = scr2.tile(
                            [128, 512], F32, name="enx", tag="enx", bufs=4
                        )
                        nc.scalar.activation(enx[:], p[:], AF.Exp, scale=-1.0)
                        enxs.append((mi, nh, enx))
                for mi, nh, enx in enxs:
                    nc.scalar.activation(
                        spT[:, mi, nh * 512:(nh + 1) * 512], enx[:],
                        AF.Ln, bias=1.0,
                    )
                # per-chunk inclusive cumsum of spT, chunk-end columns
                for mi in range(MIQ):
                    for c in range(NCH):
                        csl = slice(c * 128, (c + 1) * 128)
                        nc.vector.tensor_tensor_scan(
                            out=bsum[:, mi, csl],
                            data0=ones_sb[:],
                            data1=spT[:, mi, csl],
                            initial=0.0,
                            op0=OP.mult,
                            op1=OP.add,
                        )
                        nc.vector.tensor_scalar_mul(
                            bCn[:, mi, c:c + 1],
                            bsum[:, mi, c * 128 + 127:c * 128 + 128],
                            -1.0 / GATE_NORM,
                        )
                    nc.scalar.activation(Eall[:, mi, :], bCn[:, mi, :], AF.Exp)

                def conv_proj(w_dram, conv_sb, dst, mi_count, prio_tag):
                    """dst[:, mi, :] = silu(conv4(src @ W[:, mi-block]))^T.

                    The pre-activation is staged (padded) in SBUF via ACT
                    copies, so each conv tap is one full-span shifted
                    multiply-add; alternate tiles run on DVE vs GpSimd.
                    """
                    for mi in range(mi_count):
                        w_mi = wmi_pool.tile(
                            [128, 8, 128], F32R, name="w_mi", tag="w_mi"
                        )
                        nc.sync.dma_start(
                            out=w_mi[:],
                            in_=w_dram[:, mi * 128:(mi + 1) * 128].rearrange(
                                "(kt p) m -> p kt m", p=128
                            ),
                        )
                        pre = scr2.tile([128, 1027], F32, name="pre", tag="pre")
                        nc.gpsimd.memset(pre[:, 0:3], 0.0)
                        for nh in range(2):
                            p = ps_proj.tile(
                                [128, 512], F32, name=f"pp_{prio_tag}", tag="pp"
                            )
                            for kt in range(8):
                                nc.tensor.matmul(
                                    p[:],
                                    w_mi[:, kt, :],
                                    srcT[:, kt, nh * 512:(nh + 1) * 512],
                                    start=(kt == 0),
                                    stop=(kt == 7),
                                )
                            nc.scalar.copy(
                                out=pre[:, 3 + nh * 512:3 + (nh + 1) * 512], in_=p[:]
                            )
                        # causal conv: out[t] = sum_j pre_padded[t + j] w[:, j]
                        # (GpSimd has no TensorScalarPtr, so taps stay on DVE)
                        dst_mi = dst[:, mi, :]
                        w_of = lambda j: conv_sb[:, mi * CONV + j: mi * CONV + j + 1]
                        nc.vector.tensor_scalar_mul(dst_mi[:], pre[:, 3:1027], w_of(3))
                        for j in range(2, -1, -1):
                            nc.vector.scalar_tensor_tensor(
                                out=dst_mi[:],
                                in0=pre[:, j:j + 1024],
                                scalar=w_of(j),
                                in1=dst_mi[:],
                                op0=OP.mult,
                                op1=OP.add,
                            )
                        # silu(x) = x * sigmoid(x); multiply on idle GpSimd
                        sg = scr2.tile([128, 1024], F32, name="sg", tag="sg")
                        nc.scalar.activation(sg[:], dst_mi[:], AF.Sigmoid)
                        nc.gpsimd.tensor_mul(dst_mi[:], dst_mi[:], sg[:])

                nc.gpsimd.dma_start(
                    out=wgate_sb[:],
                    in_=wgate_d[:].rearrange("(kt p) m -> p kt m", p=128),
                )
                conv_proj(wq_d[:], convq_sb, q_sb, MIQ, "q")
                conv_proj(wk_d[:], convk_sb, k_sb, MIQ, "k")

                # q~ = q exp(b)/8 and k~ = k exp(-b), full-T, in place, with
                # f32r rounding on the write (they feed matmuls from here on).
                # Emitted right after the q/k convs so the chunk loop can start
                # while the v conv and gate projection are still running.
                for mi in range(MIQ):
                    texp = scr2.tile([128, 1024], F32, name="texp", tag="texp")
                    nc.scalar.activation(
                        texp[:], bsum[:, mi, :], AF.Exp,
                        scale=-1.0 / GATE_NORM, bias=negln8[:],
                    )
                    for half in range(2):
                        hsl = slice(half * 512, (half + 1) * 512)
                        nc.vector.tensor_mul(
                            q_sb[:, mi, hsl], q_sb[:, mi, hsl], texp[:, hsl]
                        )
                    texp2 = scr2.tile([128, 1024], F32, name="texp2", tag="texp")
                    nc.scalar.activation(
                        texp2[:], bsum[:, mi, :], AF.Exp, scale=1.0 / GATE_NORM,
                    )
                    for half in range(2):
                        hsl = slice(half * 512, (half + 1) * 512)
                        nc.vector.tensor_mul(
                            k_sb[:, mi, hsl], k_sb[:, mi, hsl], texp2[:, hsl]
                        )

                # gate: silu(src @ Wgate), natural (t-major) layout — dense PE
                # work that overlaps the conv's DVE stretch
                for mt in range(8):
                    p = ps_proj.tile([128, 512], F32, name="pp_gate", tag="pp")
                    for kt in range(8):
                        nc.tensor.matmul(
                            p[:],
                            srcT[:, kt, mt * 128:(mt + 1) * 128],
                            wgate_sb[:, kt, :],
                            start=(kt == 0),
                            stop=(kt == 7),
                        )
                    sgg = scr2.tile([128, 512], F32, name="sgg", tag="sgg")
                    nc.scalar.activation(sgg[:], p[:], AF.Sigmoid)
                    nc.vector.tensor_mul(gate_sb[:, mt, :], p[:], sgg[:])

                nc.gpsimd.dma_start(
                    out=wo_sb[:], in_=wo_d[:].rearrange("(h p) m -> p h m", p=128)
                )
                conv_proj(wv_d[:], convv_sb, v_sb, MIV, "v")

            # ---- phase 4: GLA chunk recurrence -----------------------------
            with (
                tc.tile_pool(name="ogT_pool", bufs=1) as ogT_pool,
            ):
                ogT = ogT_pool.tile([128, MIV, T], F32R, name="ogT")
                with (
                    tc.tile_pool(name="ps_h", bufs=4, space="PSUM") as ps_h,
                    tc.tile_pool(name="ps_o", bufs=2, space="PSUM") as ps_o_pool,
                ):
                    for c in range(NCH):
                        csl = slice(c * 128, (c + 1) * 128)
                        for g in range(G):
                            qt = q_sb[:, g, csl]
                            kt_ = k_sb[:, g, csl]
                            e_col = Eall[:, g, c:c + 1]
                            # k^ = k~ * exp(b_C)  (per-partition scalar)
                            kh_s = scr.tile([128, 128], F32, name="kh_s", tag="kh_s")
                            nc.vector.tensor_scalar_mul(kh_s[:], k_sb[:, g, csl], e_col)
                            nc.vector.tensor_copy(
                                out=qblk[g][0:64, 0:128], in_=qt[0:64, :]
                            )
                            nc.vector.tensor_copy(
                                out=qblk[g][64:128, 128:256], in_=qt[64:128, :]
                            )
                            # A~[s, t] for both heads: (s, [t_h0 | t_h1])
                            ps_a = ps_h.tile([128, 256], F32, name="ps_a", tag="ps_h")
                            nc.tensor.matmul(
                                ps_a[:], kt_, qblk[g][:], start=True, stop=True
                            )
                            a_sb = scr.tile([128, 256], F32R, name="a_sb", tag="a_sb")
                            nc.vector.tensor_mul(a_sb[:], ps_a[:], triu2[:])
                            # v chunk, time-major (+ padding mask)
                            ps_v = ps_h.tile([128, 256], F32, name="ps_v", tag="ps_h")
                            nc.tensor.matmul(
                                ps_v[:, 0:128], v_sb[:, 2 * g, csl], ident[:],
                                is_transpose=True, start=True, stop=False,
                                skip_group_check=True,
                            )
                            nc.tensor.matmul(
                                ps_v[:, 128:256], v_sb[:, 2 * g + 1, csl], ident[:],
                                is_transpose=True, start=False, stop=True,
                                skip_group_check=True,
                            )
                            vnat = scr.tile([128, 256], F32R, name="vnat", tag="vnat")
                            nc.vector.tensor_scalar_mul(
                                vnat[:], ps_v[:], maskc_sb[:, c:c + 1]
                            )
                            # k^ chunk, time-major
                            ps_k = ps_t.tile([128, 256], F32, name="ps_k", tag="pst")
                            nc.tensor.transpose(ps_k[:, 0:128], kh_s[:], ident[:])
                            khnat = scr.tile([128, 128], F32R, name="khnat", tag="khnat")
                            nc.scalar.copy(out=khnat[:], in_=ps_k[:, 0:128])
                            # o = A~^T v (intra) + q~ @ S (inter)
                            ps_o = ps_o_pool.tile([128, 256], F32, name="ps_o", tag="ps_o")
                            nc.tensor.matmul(
                                ps_o[:, 0:128], a_sb[:, 0:128], vnat[:, 0:128],
                                start=True, stop=False, skip_group_check=True,
                            )
                            nc.tensor.matmul(
                                ps_o[:, 128:256], a_sb[:, 128:256], vnat[:, 128:256],
                                start=False, stop=False, skip_group_check=True,
                            )
                            nc.tensor.matmul(
                                ps_o[:], qt, Sblk[g][:],
                                start=False, stop=True, skip_group_check=True,
                            )
                            # state update: S = diag(exp(b_C)) S + k^T v
                            ps_s = ps_h.tile([128, 256], F32, name="ps_s", tag="ps_h")
                            nc.tensor.matmul(
                                ps_s[:], khnat[:], vnat[:], start=True, stop=True
                            )
                            nc.vector.scalar_tensor_tensor(
                                out=Sblk[g][0:64, 0:128],
                                in0=Sblk[g][0:64, 0:128],
                                scalar=e_col[0:64, :],
                                in1=ps_s[0:64, 0:128],
                                op0=OP.mult,
                                op1=OP.add,
                            )
                            nc.vector.scalar_tensor_tensor(
                                out=Sblk[g][64:128, 128:256],
                                in0=Sblk[g][64:128, 128:256],
                                scalar=e_col[64:128, :],
                                in1=ps_s[64:128, 128:256],
                                op0=OP.mult,
                                op1=OP.add,
                            )
                            # evacuate o: multiply the swish gate in-place into
                            # gate_sb, and collect per-head sums of squares
                            for lh in range(2):
                                sqd = scr.tile([128, 128], F32, name="sqd", tag="sqd")
                                idx = c * 4 + 2 * g + lh
                                nc.scalar.activation(
                                    sqd[:], ps_o[:, lh * 128:(lh + 1) * 128],
                                    AF.Square,
                                    accum_out=ssq_all[:, idx:idx + 1],
                                )
                            gsl = slice(g * 256, (g + 1) * 256)
                            nc.vector.tensor_mul(
                                gate_sb[:, c, gsl], ps_o[:], gate_sb[:, c, gsl]
                            )

                # ---- tail: rrms, gate-scale, transpose, and the output
                # projection all pipelined per chunk (chunk c is exactly
                # output row-tile mt=c, so each chunk streams straight
                # through Wo and out to DRAM)
                lnr = scr.tile([128, NCH * 4], F32, name="lnr", tag="lnr")
                nc.scalar.activation(
                    lnr[:], ssq_all[:], AF.Ln, scale=1.0 / DV, bias=eps_col[:]
                )
                nc.scalar.activation(rrms_all[:], lnr[:], AF.Exp, scale=-0.5)
                with (
                    tc.tile_pool(name="ps_out", bufs=3, space="PSUM") as ps_out,
                    tc.tile_pool(name="stage", bufs=3) as stage_pool,
                ):
                    for c in range(NCH):
                        csl = slice(c * 128, (c + 1) * 128)
                        rr = rrms_all[:, c * 4:(c + 1) * 4, None].to_broadcast(
                            (128, 4, 128)
                        )
                        nc.vector.tensor_mul(
                            gate_sb[:, c, :].rearrange("p (h x) -> p h x", h=4),
                            gate_sb[:, c, :].rearrange("p (h x) -> p h x", h=4),
                            rr,
                        )
                        for h in range(0, 4, 2):
                            ps_g = ps_t.tile([128, 256], F32, name="ps_g", tag="pst")
                            nc.tensor.matmul(
                                ps_g[:, 0:128], gate_sb[:, c, h * 128:(h + 1) * 128],
                                ident[:], is_transpose=True, start=True, stop=False,
                                skip_group_check=True,
                            )
                            nc.tensor.matmul(
                                ps_g[:, 128:256],
                                gate_sb[:, c, (h + 1) * 128:(h + 2) * 128],
                                ident[:], is_transpose=True, start=False, stop=True,
                                skip_group_check=True,
                            )
                            nc.scalar.copy(
                                out=ogT[:, h:h + 2, csl],
                                in_=ps_g[:].rearrange("p (a b) -> p a b", a=2),
                            )
                        for nh in range(2):
                            p = ps_out.tile([128, 512], F32, name="p_out", tag="p_out")
                            for h in range(4):
                                nc.tensor.matmul(
                                    p[:],
                                    ogT[:, h, csl],
                                    wo_sb[:, h, nh * 512:(nh + 1) * 512],
                                    start=(h == 0),
                                    stop=(h == 3),
                                )
                            stage = stage_pool.tile(
                                [128, 512], F32, name="stage", tag="stage"
                            )
                            if (c + nh) % 2 == 0:
                                nc.vector.tensor_copy(out=stage[:], in_=p[:])
                            else:
                                nc.scalar.copy(out=stage[:], in_=p[:])
                            nc.gpsimd.dma_start(
                                out=out_d[c * 128:(c + 1) * 128,
                                          nh * 512:(nh + 1) * 512],
                                in_=stage[:],
                            )

    nc.compile()
    return nc


_NC_CACHE = None


def _get_program():
    global _NC_CACHE
    if _NC_CACHE is None:
        _NC_CACHE = build_program()
    return _NC_CACHE


def shard_inputs(
    src, valid_mask, Wq, Wk, Wv, conv_q_w, conv_k_w, conv_v_w,
    Wg1, Wg2, bg2, Wgate, rms_w, Wo,
):
    """Build the 8 per-core input maps."""
    f = np.float32
    src = np.asarray(src, f)
    valid_mask = np.asarray(valid_mask)
    in_maps = []
    wo_scaled = np.asarray(Wo, f) * np.tile(np.asarray(rms_w, f), VD // DV)[:, None]
    for core in range(NCORES):
        b, hg = core // 2, core % 2
        qs = slice(hg * KDC, (hg + 1) * KDC)
        vs = slice(hg * VDC, (hg + 1) * VDC)
        wg2b = np.concatenate(
            [np.asarray(Wg2, f)[:, qs], np.asarray(bg2, f)[None, qs]], axis=0
        )

        def conv_fold(w, mi_count):
            w = np.asarray(w, f)  # (chans, 4) slice for this core
            return np.ascontiguousarray(
                w.reshape(mi_count, 128, CONV).transpose(1, 0, 2).reshape(128, -1)
            )

        in_maps.append({
            "srcT_in": np.ascontiguousarray(src[b].T),
            "wq": np.ascontiguousarray(np.asarray(Wq, f)[:, qs]),
            "wk": np.ascontiguousarray(np.asarray(Wk, f)[:, qs]),
            "wv": np.ascontiguousarray(np.asarray(Wv, f)[:, vs]),
            "wgate": np.ascontiguousarray(np.asarray(Wgate, f)[:, vs]),
            "wg1": np.ascontiguousarray(np.asarray(Wg1, f)),
            "wg2b": np.ascontiguousarray(wg2b),
            "wo": np.ascontiguousarray(wo_scaled[vs, :]),
            "convq": conv_fold(np.asarray(conv_q_w, f)[qs], MIQ),
            "convk": conv_fold(np.asarray(conv_k_w, f)[qs], MIQ),
            "convv": conv_fold(np.asarray(conv_v_w, f)[vs], MIV),
            "maskc": np.ascontiguousarray(
                valid_mask[b].astype(f).reshape(NCH, 128).T
            ),
        })
    return in_maps


def kernel(**inputs):
    nc = _get_program()
    in_maps = shard_inputs(**inputs)
    res = run_bass_kernel_spmd(nc, in_maps, list(range(NCORES)))
    out = np.zeros((B, T, D), np.float32)
    for core in range(NCORES):
        out[core // 2] += res.results[core]["out"]
    return out


if __name__ == "__main__":
    prog = _get_program()
    print("program built OK")


# revision 51
# speedup vs baseline: 1.0604x; 1.0265x over previous
"""Gated Linear Attention forward on 8 Trainium2 NeuronCores (Bass/Tile).

Problem: B=4, T=1024, D=1024, H=8, DK=64, DV=128, conv4 on q/k/v, low-rank
log-sigmoid forget gate, recurrent scan, RMS-norm + swish output gate, out proj.

Sharding: core = 2*b + hg  (b = batch, hg = half of the heads).
Each core computes its batch's tokens for 4 heads end-to-end and a partial
output projection (Wo row-block); the host sums the two partials per batch.

On-device algorithm: chunked-parallel GLA with chunk C=128.
Per chunk (local inclusive cumsum b of the log-gates):
  q~ = q * exp(b)/8,  k~ = k * exp(-b),  k^ = k~ * exp(b_C)
  A~[s,t] = sum_kk k~[s] q~[t]   masked to s<=t
  o = A~^T v (intra) + q~ @ S (inter), accumulated in one PSUM tile
  S' = diag(exp(b_C)) S + k^T v
Layouts: projections are computed transposed (channels on partitions, time on
free) so the depthwise conv is a per-partition-scalar shifted multiply-add;
v / k^ / gated-o are PE-transposed per 128x128 tile where time must sit on
partitions. Matmuls run as float32r (fp22 multiplies, fp32 accumulation); the
producers of every matmul operand write with f32r rounding to satisfy the BIR
verifier. q~/k~ are formed in place over the full T once the cumsum is ready.
The RMS rsqrt is deferred to a single Ln+Exp pass after the chunk loop so the
ACT engine never thrashes activation-table loads inside the loop.
"""

import numpy as np

import concourse.bass as bass
import concourse.mybir as mybir
import concourse.tile as tile
from concourse import bacc
from concourse.bass_utils import run_bass_kernel_spmd

F32 = mybir.dt.float32
F32R = mybir.dt.float32r
AF = mybir.ActivationFunctionType
OP = mybir.AluOpType

# problem constants (hardcoded per the task contract)
B, T, D, H = 4, 1024, 1024, 8
KD, VD = 512, 1024
DK, DV = 64, 128
CONV = 4
GATE_NORM = 16.0
EPS = 1e-5
LN8 = float(np.log(8.0))

# per-core shapes
KDC, VDC = 256, 512          # q/k and v/gate channels per core
MIQ, MIV = 2, 4              # 128-wide channel tiles for q/k and v
C, NCH = 128, 8              # chunk length, number of chunks
G = 2                        # head groups of 2 heads (128 chans) per core
NCORES = 8


def build_program():
    nc = bacc.Bacc("TRN2", target_bir_lowering=False, debug=False)

    # ---- DRAM I/O ----------------------------------------------------------
    srcT_d = nc.dram_tensor("srcT_in", [D, T], F32R, kind="ExternalInput")
    wq_d = nc.dram_tensor("wq", [D, KDC], F32R, kind="ExternalInput")
    wk_d = nc.dram_tensor("wk", [D, KDC], F32R, kind="ExternalInput")
    wv_d = nc.dram_tensor("wv", [D, VDC], F32R, kind="ExternalInput")
    wgate_d = nc.dram_tensor("wgate", [D, VDC], F32R, kind="ExternalInput")
    wg1_d = nc.dram_tensor("wg1", [D, 16], F32R, kind="ExternalInput")
    wg2b_d = nc.dram_tensor("wg2b", [17, KDC], F32R, kind="ExternalInput")
    wo_d = nc.dram_tensor("wo", [VDC, D], F32R, kind="ExternalInput")
    convdiag_d = nc.dram_tensor(
        "convdiag", [(2 * MIQ + MIV) * CONV, 128, 128], F32R, kind="ExternalInput"
    )
    maskc_d = nc.dram_tensor("maskc", [128, NCH], F32, kind="ExternalInput")
    out_d = nc.dram_tensor("out", [T, D], F32, kind="ExternalOutput")

    ident_np = np.eye(128, dtype=np.float32)
    u = np.triu(np.ones((128, 128), np.float32))  # U[s,t] = 1 iff s <= t
    ident_d = nc.inline_tensor(ident_np, "ident_c")
    triu2_d = nc.inline_tensor(np.concatenate([u, u], axis=1), "triu2_c")

    # ---- static SBUF -------------------------------------------------------
    srcT = nc.alloc_sbuf_tensor("srcT", [128, 8, T], F32R)      # src^T, d-major
    q_sb = nc.alloc_sbuf_tensor("q_sb", [128, MIQ, T], F32R)     # q then q~ (in place)
    k_sb = nc.alloc_sbuf_tensor("k_sb", [128, MIQ, T], F32R)     # k then k~
    v_sb = nc.alloc_sbuf_tensor("v_sb", [128, MIV, T], F32)
    gate_sb = nc.alloc_sbuf_tensor("gate_sb", [128, NCH, VDC], F32)  # silu(gate) → o*gate
    xgT = nc.alloc_sbuf_tensor("xgT", [17, T], F32R)            # (src@Wg1)^T + ones row
    spT = nc.alloc_sbuf_tensor("spT", [128, MIQ, T], F32)       # softplus(-gk_logit)
    bsum = nc.alloc_sbuf_tensor("bsum", [128, MIQ, T], F32)     # per-chunk cumsum of spT
    bCn = nc.alloc_sbuf_tensor("bCn", [128, MIQ, NCH], F32)     # -spsum_last/16 per chunk
    ssq_all = nc.alloc_sbuf_tensor("ssq_all", [128, NCH * 4], F32)   # col = c*4 + head
    rrms_all = nc.alloc_sbuf_tensor("rrms_all", [128, NCH * 4], F32)
    Eall = nc.alloc_sbuf_tensor("Eall", [128, MIQ, NCH], F32)   # exp(b_C) per chunk
    wo_sb = nc.alloc_sbuf_tensor("wo_sb", [128, MIV, D], F32R)
    wgate_sb = nc.alloc_sbuf_tensor("wgate_sb", [128, 8, VDC], F32R)
    wg1_sb = nc.alloc_sbuf_tensor("wg1_sb", [128, 8, 16], F32R)
    wg2b_sb = nc.alloc_sbuf_tensor("wg2b_sb", [17, KDC], F32R)
    maskc_sb = nc.alloc_sbuf_tensor("maskc_sb", [128, NCH], F32)
    ident = nc.alloc_sbuf_tensor("ident", [128, 128], F32)
    triu2 = nc.alloc_sbuf_tensor("triu2", [128, 256], F32)
    ones_sb = nc.alloc_sbuf_tensor("ones_sb", [128, 128], F32)
    Sblk = [nc.alloc_sbuf_tensor(f"Sblk{g}", [128, 256], F32R) for g in range(G)]
    qblk = [nc.alloc_sbuf_tensor(f"qblk{g}", [128, 256], F32R) for g in range(G)]
    negln8 = nc.alloc_sbuf_tensor("negln8", [128, 1], F32)
    eps_col = nc.alloc_sbuf_tensor("eps_col", [128, 1], F32)

    with tile.TileContext(nc) as tc:
        with (
            tc.tile_pool(name="scr", bufs=4) as scr,
            tc.tile_pool(name="ps_t", bufs=2, space="PSUM") as ps_t,
        ):
            # ---- phase 0: constants in (src streams first; the big late
            # weights go on the GpSimd DMA queue so they don't delay src) ----
            nc.sync.dma_start(out=ident[:], in_=ident_d[:])
            nc.sync.dma_start(out=triu2[:], in_=triu2_d[:])
            nc.sync.dma_start(out=maskc_sb[:], in_=maskc_d[:])
            nc.sync.dma_start(
                out=wg1_sb[:], in_=wg1_d[:].rearrange("(kt p) m -> p kt m", p=128)
            )
            nc.sync.dma_start(out=wg2b_sb[:], in_=wg2b_d[:])
            nc.vector.memset(ones_sb[:], 1.0)
            # row 16 must be ones (bias row); rows 0..15 are overwritten later
            nc.vector.memset(xgT[:].bitcast(F32), 1.0)
            nc.vector.memset(negln8[:], -LN8)
            nc.vector.memset(eps_col[:], EPS)
            for g in range(G):
                nc.vector.memset(Sblk[g][:].bitcast(F32), 0.0)
                nc.vector.memset(qblk[g][:].bitcast(F32), 0.0)

            # ---- phases 1+2: gk; projections+conv; gate --------------------
            # (src arrives pre-transposed from the host: one straight DMA)
            with (
                tc.tile_pool(name="wmi", bufs=4) as wmi_pool,
                tc.tile_pool(name="scr2", bufs=2) as scr2,
                tc.tile_pool(name="ps_proj", bufs=6, space="PSUM") as ps_proj,
            ):
                # eight separate DMAs, issued from all three DMA-capable
                # engines so the 4MB spreads across independent DMA paths
                dma_engs = [nc.sync, nc.scalar, nc.gpsimd]
                for kt in range(8):
                    dma_engs[kt % 3].dma_start(
                        out=srcT[:, kt, :],
                        in_=srcT_d[kt * 128:(kt + 1) * 128, :],
                    )

                # gk path first so the chunk loop's inputs are ready early:
                # xg^T = (src @ Wg1)^T with an appended ones row
                for nh in range(2):
                    p = ps_proj.tile([128, 512], F32, name="pp_xg", tag="pp")
                    for kt in range(8):
                        nc.tensor.matmul(
                            p[0:16, :],
                            wg1_sb[:, kt, :],
                            srcT[:, kt, nh * 512:(nh + 1) * 512],
                            start=(kt == 0),
                            stop=(kt == 7),
                        )
                    nc.vector.tensor_copy(
                        out=xgT[0:16, nh * 512:(nh + 1) * 512], in_=p[0:16, :]
                    )
                # spT = softplus(-(xg @ Wg2 + bg2)) = log(1 + exp(-logit))
                # (logits here are O(1), so exp(-x) cannot overflow).
                # All 4 Exps run before all 4 Lns so the ACT engine loads
                # each activation table once instead of thrashing.
                enxs = []
                for mi in range(MIQ):
                    for nh in range(2):
                        p = ps_proj.tile([128, 512], F32, name="pp_sp", tag="pp")
                        nc.tensor.matmul(
                            p[:],
                            wg2b_sb[:, mi * 128:(mi + 1) * 128],
                            xgT[:, nh * 512:(nh + 1) * 512],
                            start=True,
                            stop=True,
                        )
                        enx = scr2.tile(
                            [128, 512], F32, name="enx", tag="enx", bufs=4
                        )
                        nc.scalar.activation(enx[:], p[:], AF.Exp, scale=-1.0)
                        enxs.append((mi, nh, enx))
                for mi, nh, enx in enxs:
                    nc.scalar.activation(
                        spT[:, mi, nh * 512:(nh + 1) * 512], enx[:],
                        AF.Ln, bias=1.0,
                    )
                # per-chunk inclusive cumsum of spT, chunk-end columns
                for mi in range(MIQ):
                    for c in range(NCH):
                        csl = slice(c * 128, (c + 1) * 128)
                        nc.vector.tensor_tensor_scan(
                            out=bsum[:, mi, csl],
                            data0=ones_sb[:],
                            data1=spT[:, mi, csl],
                            initial=0.0,
                            op0=OP.mult,
                            op1=OP.add,
                        )
                        nc.vector.tensor_scalar_mul(
                            bCn[:, mi, c:c + 1],
                            bsum[:, mi, c * 128 + 127:c * 128 + 128],
                            -1.0 / GATE_NORM,
                        )
                    nc.scalar.activation(Eall[:, mi, :], bCn[:, mi, :], AF.Exp)

                def conv_proj(w_dram, diag_base, dst, mi_count, prio_tag):
                    """dst[:, mi, :] = silu(conv4(src @ W[:, mi-block]))^T.

                    The pre-activation is staged (padded) in SBUF via ACT
                    copies; the conv itself runs on the PE as 4 shifted
                    diagonal-matrix matmuls accumulating in PSUM (the DVE was
                    the kernel-wide bottleneck, the PE has slack here).
                    """
                    for mi in range(mi_count):
                        w_mi = wmi_pool.tile(
                            [128, 8, 128], F32R, name="w_mi", tag="w_mi"
                        )
                        nc.sync.dma_start(
                            out=w_mi[:],
                            in_=w_dram[:, mi * 128:(mi + 1) * 128].rearrange(
                                "(kt p) m -> p kt m", p=128
                            ),
                        )
                        dg = wmi_pool.tile([128, 4, 128], F32R, name="dg", tag="dg")
                        nc.scalar.dma_start(
                            out=dg[:],
                            in_=convdiag_d[4 * (diag_base + mi):
                                           4 * (diag_base + mi) + 4].rearrange(
                                "j p m -> p j m"
                            ),
                        )
                        pre = scr2.tile([128, 1027], F32R, name="pre", tag="pre")
                        nc.gpsimd.memset(pre[:, 0:3].bitcast(F32), 0.0)
                        for nh in range(2):
                            p = ps_proj.tile(
                                [128, 512], F32, name=f"pp_{prio_tag}", tag="pp"
                            )
                            for kt in range(8):
                                nc.tensor.matmul(
                                    p[:],
                                    w_mi[:, kt, :],
                                    srcT[:, kt, nh * 512:(nh + 1) * 512],
                                    start=(kt == 0),
                                    stop=(kt == 7),
                                )
                            nc.scalar.copy(
                                out=pre[:, 3 + nh * 512:3 + (nh + 1) * 512], in_=p[:]
                            )
                        # causal conv: out[t] = sum_j pre_padded[t + j] w[:, j]
                        # as 4 diag(w_j) matmuls with shifted moving operands
                        dst_mi = dst[:, mi, :]
                        for nh in range(2):
                            cp = ps_proj.tile(
                                [128, 512], F32, name=f"cp_{prio_tag}", tag="pp"
                            )
                            for j in range(CONV):
                                nc.tensor.matmul(
                                    cp[:],
                                    dg[:, j, :],
                                    pre[:, nh * 512 + j:nh * 512 + j + 512],
                                    start=(j == 0),
                                    stop=(j == 3),
                                )
                            # silu(x) = x * sigmoid(x), evacuating the PSUM
                            sg = scr2.tile([128, 512], F32, name="sg", tag="sg")
                            nc.scalar.activation(sg[:], cp[:], AF.Sigmoid)
                            nc.vector.tensor_mul(
                                dst_mi[:, nh * 512:(nh + 1) * 512], cp[:], sg[:]
                            )

                nc.gpsimd.dma_start(
                    out=wgate_sb[:],
                    in_=wgate_d[:].rearrange("(kt p) m -> p kt m", p=128),
                )
                conv_proj(wq_d[:], 0, q_sb, MIQ, "q")
                conv_proj(wk_d[:], MIQ, k_sb, MIQ, "k")

                # q~ = q exp(b)/8 and k~ = k exp(-b), full-T, in place, with
                # f32r rounding on the write (they feed matmuls from here on).
                # Emitted right after the q/k convs so the chunk loop can start
                # while the v conv and gate projection are still running.
                for mi in range(MIQ):
                    texp = scr2.tile([128, 1024], F32, name="texp", tag="texp", bufs=1)
                    nc.scalar.activation(
                        texp[:], bsum[:, mi, :], AF.Exp,
                        scale=-1.0 / GATE_NORM, bias=negln8[:],
                    )
                    for half in range(2):
                        hsl = slice(half * 512, (half + 1) * 512)
                        nc.vector.tensor_mul(
                            q_sb[:, mi, hsl], q_sb[:, mi, hsl], texp[:, hsl]
                        )
                    texp2 = scr2.tile([128, 1024], F32, name="texp2", tag="texp", bufs=1)
                    nc.scalar.activation(
                        texp2[:], bsum[:, mi, :], AF.Exp, scale=1.0 / GATE_NORM,
                    )
                    for half in range(2):
                        hsl = slice(half * 512, (half + 1) * 512)
                        nc.vector.tensor_mul(
                            k_sb[:, mi, hsl], k_sb[:, mi, hsl], texp2[:, hsl]
                        )

                # gate: silu(src @ Wgate), natural (t-major) layout — dense PE
                # work that overlaps the conv's DVE stretch
                for mt in range(8):
                    p = ps_proj.tile([128, 512], F32, name="pp_gate", tag="pp")
                    for kt in range(8):
                        nc.tensor.matmul(
                            p[:],
                            srcT[:, kt, mt * 128:(mt + 1) * 128],
                            wgate_sb[:, kt, :],
                            start=(kt == 0),
                            stop=(kt == 7),
                        )
                    sgg = scr2.tile([128, 512], F32, name="sgg", tag="sgg")
                    nc.scalar.activation(sgg[:], p[:], AF.Sigmoid)
                    nc.vector.tensor_mul(gate_sb[:, mt, :], p[:], sgg[:])

                nc.gpsimd.dma_start(
                    out=wo_sb[:], in_=wo_d[:].rearrange("(h p) m -> p h m", p=128)
                )
                conv_proj(wv_d[:], 2 * MIQ, v_sb, MIV, "v")

            # ---- phase 4: GLA chunk recurrence -----------------------------
            with (
                tc.tile_pool(name="ogT_pool", bufs=1) as ogT_pool,
            ):
                ogT = ogT_pool.tile([128, MIV, T], F32R, name="ogT")
                with (
                    tc.tile_pool(name="ps_h", bufs=4, space="PSUM") as ps_h,
                    tc.tile_pool(name="ps_o", bufs=2, space="PSUM") as ps_o_pool,
                ):
                    for c in range(NCH):
                        csl = slice(c * 128, (c + 1) * 128)
                        for g in range(G):
                            qt = q_sb[:, g, csl]
                            kt_ = k_sb[:, g, csl]
                            e_col = Eall[:, g, c:c + 1]
                            # k^ = k~ * exp(b_C)  (per-partition scalar)
                            kh_s = scr.tile([128, 128], F32, name="kh_s", tag="kh_s")
                            nc.vector.tensor_scalar_mul(kh_s[:], k_sb[:, g, csl], e_col)
                            nc.vector.tensor_copy(
                                out=qblk[g][0:64, 0:128], in_=qt[0:64, :]
                            )
                            nc.vector.tensor_copy(
                                out=qblk[g][64:128, 128:256], in_=qt[64:128, :]
                            )
                            # A~[s, t] for both heads: (s, [t_h0 | t_h1])
                            ps_a = ps_h.tile([128, 256], F32, name="ps_a", tag="ps_h")
                            nc.tensor.matmul(
                                ps_a[:], kt_, qblk[g][:], start=True, stop=True
                            )
                            a_sb = scr.tile([128, 256], F32R, name="a_sb", tag="a_sb")
                            nc.vector.tensor_mul(a_sb[:], ps_a[:], triu2[:])
                            # v chunk, time-major (+ padding mask)
                            ps_v = ps_h.tile([128, 256], F32, name="ps_v", tag="ps_h")
                            nc.tensor.matmul(
                                ps_v[:, 0:128], v_sb[:, 2 * g, csl], ident[:],
                                is_transpose=True, start=True, stop=False,
                                skip_group_check=True,
                            )
                            nc.tensor.matmul(
                                ps_v[:, 128:256], v_sb[:, 2 * g + 1, csl], ident[:],
                                is_transpose=True, start=False, stop=True,
                                skip_group_check=True,
                            )
                            vnat = scr.tile([128, 256], F32R, name="vnat", tag="vnat")
                            nc.vector.tensor_scalar_mul(
                                vnat[:], ps_v[:], maskc_sb[:, c:c + 1]
                            )
                            # k^ chunk, time-major
                            ps_k = ps_t.tile([128, 256], F32, name="ps_k", tag="pst")
                            nc.tensor.transpose(ps_k[:, 0:128], kh_s[:], ident[:])
                            khnat = scr.tile([128, 128], F32R, name="khnat", tag="khnat")
                            nc.scalar.copy(out=khnat[:], in_=ps_k[:, 0:128])
                            # o = A~^T v (intra) + q~ @ S (inter)
                            ps_o = ps_o_pool.tile([128, 256], F32, name="ps_o", tag="ps_o")
                            nc.tensor.matmul(
                                ps_o[:, 0:128], a_sb[:, 0:128], vnat[:, 0:128],
                                start=True, stop=False, skip_group_check=True,
                            )
                            nc.tensor.matmul(
                                ps_o[:, 128:256], a_sb[:, 128:256], vnat[:, 128:256],
                                start=False, stop=False, skip_group_check=True,
                            )
                            nc.tensor.matmul(
                                ps_o[:], qt, Sblk[g][:],
                                start=False, stop=True, skip_group_check=True,
                            )
                            # state update: S = diag(exp(b_C)) S + k^T v
                            ps_s = ps_h.tile([128, 256], F32, name="ps_s", tag="ps_h")
                            nc.tensor.matmul(
                                ps_s[:], khnat[:], vnat[:], start=True, stop=True
                            )
                            nc.vector.scalar_tensor_tensor(
                                out=Sblk[g][0:64, 0:128],
                                in0=Sblk[g][0:64, 0:128],
                                scalar=e_col[0:64, :],
                                in1=ps_s[0:64, 0:128],
                                op0=OP.mult,
                                op1=OP.add,
                            )
                            nc.vector.scalar_tensor_tensor(
                                out=Sblk[g][64:128, 128:256],
                                in0=Sblk[g][64:128, 128:256],
                                scalar=e_col[64:128, :],
                                in1=ps_s[64:128, 128:256],
                                op0=OP.mult,
                                op1=OP.add,
                            )
                            # evacuate o: multiply the swish gate in-place into
                            # gate_sb, and collect per-head sums of squares
                            for lh in range(2):
                                sqd = scr.tile([128, 128], F32, name="sqd", tag="sqd")
                                idx = c * 4 + 2 * g + lh
                                nc.scalar.activation(
                                    sqd[:], ps_o[:, lh * 128:(lh + 1) * 128],
                                    AF.Square,
                                    accum_out=ssq_all[:, idx:idx + 1],
                                )
                            gsl = slice(g * 256, (g + 1) * 256)
                            nc.vector.tensor_mul(
                                gate_sb[:, c, gsl], ps_o[:], gate_sb[:, c, gsl]
                            )

                # ---- tail: rrms, gate-scale, transpose, and the output
                # projection all pipelined per chunk (chunk c is exactly
                # output row-tile mt=c, so each chunk streams straight
                # through Wo and out to DRAM)
                lnr = scr.tile([128, NCH * 4], F32, name="lnr", tag="lnr")
                nc.scalar.activation(
                    lnr[:], ssq_all[:], AF.Ln, scale=1.0 / DV, bias=eps_col[:]
                )
                nc.scalar.activation(rrms_all[:], lnr[:], AF.Exp, scale=-0.5)
                with (
                    tc.tile_pool(name="ps_out", bufs=3, space="PSUM") as ps_out,
                    tc.tile_pool(name="stage", bufs=3) as stage_pool,
                ):
                    for c in range(NCH):
                        csl = slice(c * 128, (c + 1) * 128)
                        rr = rrms_all[:, c * 4:(c + 1) * 4, None].to_broadcast(
                            (128, 4, 128)
                        )
                        nc.vector.tensor_mul(
                            gate_sb[:, c, :].rearrange("p (h x) -> p h x", h=4),
                            gate_sb[:, c, :].rearrange("p (h x) -> p h x", h=4),
                            rr,
                        )
                        for h in range(0, 4, 2):
                            ps_g = ps_t.tile([128, 256], F32, name="ps_g", tag="pst")
                            nc.tensor.matmul(
                                ps_g[:, 0:128], gate_sb[:, c, h * 128:(h + 1) * 128],
                                ident[:], is_transpose=True, start=True, stop=False,
                                skip_group_check=True,
                            )
                            nc.tensor.matmul(
                                ps_g[:, 128:256],
                                gate_sb[:, c, (h + 1) * 128:(h + 2) * 128],
                                ident[:], is_transpose=True, start=False, stop=True,
                                skip_group_check=True,
                            )
                            nc.scalar.copy(
                                out=ogT[:, h:h + 2, csl],
                                in_=ps_g[:].rearrange("p (a b) -> p a b", a=2),
                            )
                        for nh in range(2):
                            p = ps_out.tile([128, 512], F32, name="p_out", tag="p_out")
                            for h in range(4):
                                nc.tensor.matmul(
                                    p[:],
                                    ogT[:, h, csl],
                                    wo_sb[:, h, nh * 512:(nh + 1) * 512],
                                    start=(h == 0),
                                    stop=(h == 3),
                                )
                            stage = stage_pool.tile(
                                [128, 512], F32, name="stage", tag="stage"
                            )
                            if (c + nh) % 2 == 0:
                                nc.vector.tensor_copy(out=stage[:], in_=p[:])
                            else:
                                nc.scalar.copy(out=stage[:], in_=p[:])
                            nc.gpsimd.dma_start(
                                out=out_d[c * 128:(c + 1) * 128,
                                          nh * 512:(nh + 1) * 512],
                                in_=stage[:],
                            )

    nc.compile()
    return nc


_NC_CACHE = None


def _get_program():
    global _NC_CACHE
    if _NC_CACHE is None:
        _NC_CACHE = build_program()
    return _NC_CACHE


def shard_inputs(
    src, valid_mask, Wq, Wk, Wv, conv_q_w, conv_k_w, conv_v_w,
    Wg1, Wg2, bg2, Wgate, rms_w, Wo,
):
    """Build the 8 per-core input maps."""
    f = np.float32
    src = np.asarray(src, f)
    valid_mask = np.asarray(valid_mask)
    in_maps = []
    wo_scaled = np.asarray(Wo, f) * np.tile(np.asarray(rms_w, f), VD // DV)[:, None]
    for core in range(NCORES):
        b, hg = core // 2, core % 2
        qs = slice(hg * KDC, (hg + 1) * KDC)
        vs = slice(hg * VDC, (hg + 1) * VDC)
        wg2b = np.concatenate(
            [np.asarray(Wg2, f)[:, qs], np.asarray(bg2, f)[None, qs]], axis=0
        )

        # one (128,128) diagonal matrix per (channel-tile, tap) for the
        # PE-side depthwise conv: q tiles, k tiles, then v tiles
        conv_diag = np.zeros(((2 * MIQ + MIV) * CONV, 128, 128), f)
        tiles = []
        for w, sel, n in ((conv_q_w, qs, MIQ), (conv_k_w, qs, MIQ),
                          (conv_v_w, vs, MIV)):
            wa = np.asarray(w, f)[sel]
            tiles.extend(wa[i * 128:(i + 1) * 128] for i in range(n))
        for ti, wt in enumerate(tiles):      # wt: (128, 4)
            for j in range(CONV):
                np.fill_diagonal(conv_diag[ti * CONV + j], wt[:, j])

        in_maps.append({
            "srcT_in": np.ascontiguousarray(src[b].T),
            "wq": np.ascontiguousarray(np.asarray(Wq, f)[:, qs]),
            "wk": np.ascontiguousarray(np.asarray(Wk, f)[:, qs]),
            "wv": np.ascontiguousarray(np.asarray(Wv, f)[:, vs]),
            "wgate": np.ascontiguousarray(np.asarray(Wgate, f)[:, vs]),
            "wg1": np.ascontiguousarray(np.asarray(Wg1, f)),
            "wg2b": np.ascontiguousarray(wg2b),
            "wo": np.ascontiguousarray(wo_scaled[vs, :]),
            "convdiag": conv_diag,
            "maskc": np.ascontiguousarray(
                valid_mask[b].astype(f).reshape(NCH, 128).T
            ),
        })
    return in_maps


def kernel(**inputs):
    nc = _get_program()
    in_maps = shard_inputs(**inputs)
    res = run_bass_kernel_spmd(nc, in_maps, list(range(NCORES)))
    out = np.zeros((B, T, D), np.float32)
    for core in range(NCORES):
        out[core // 2] += res.results[core]["out"]
    return out


if __name__ == "__main__":
    prog = _get_program()
    print("program built OK")


# revision 52
# speedup vs baseline: 1.0718x; 1.0107x over previous
"""Gated Linear Attention forward on 8 Trainium2 NeuronCores (Bass/Tile).

Problem: B=4, T=1024, D=1024, H=8, DK=64, DV=128, conv4 on q/k/v, low-rank
log-sigmoid forget gate, recurrent scan, RMS-norm + swish output gate, out proj.

Sharding: core = 2*b + hg  (b = batch, hg = half of the heads).
Each core computes its batch's tokens for 4 heads end-to-end and a partial
output projection (Wo row-block); the host sums the two partials per batch.

On-device algorithm: chunked-parallel GLA with chunk C=128.
Per chunk (local inclusive cumsum b of the log-gates):
  q~ = q * exp(b)/8,  k~ = k * exp(-b),  k^ = k~ * exp(b_C)
  A~[s,t] = sum_kk k~[s] q~[t]   masked to s<=t
  o = A~^T v (intra) + q~ @ S (inter), accumulated in one PSUM tile
  S' = diag(exp(b_C)) S + k^T v
Layouts: projections are computed transposed (channels on partitions, time on
free) so the depthwise conv is a per-partition-scalar shifted multiply-add;
v / k^ / gated-o are PE-transposed per 128x128 tile where time must sit on
partitions. Matmuls run as float32r (fp22 multiplies, fp32 accumulation); the
producers of every matmul operand write with f32r rounding to satisfy the BIR
verifier. q~/k~ are formed in place over the full T once the cumsum is ready.
The RMS rsqrt is deferred to a single Ln+Exp pass after the chunk loop so the
ACT engine never thrashes activation-table loads inside the loop.
"""

import numpy as np

import concourse.bass as bass
import concourse.mybir as mybir
import concourse.tile as tile
from concourse import bacc
from concourse.bass_utils import run_bass_kernel_spmd

F32 = mybir.dt.float32
F32R = mybir.dt.float32r
AF = mybir.ActivationFunctionType
OP = mybir.AluOpType

# problem constants (hardcoded per the task contract)
B, T, D, H = 4, 1024, 1024, 8
KD, VD = 512, 1024
DK, DV = 64, 128
CONV = 4
GATE_NORM = 16.0
EPS = 1e-5
LN8 = float(np.log(8.0))

# per-core shapes
KDC, VDC = 256, 512          # q/k and v/gate channels per core
MIQ, MIV = 2, 4              # 128-wide channel tiles for q/k and v
C, NCH = 128, 8              # chunk length, number of chunks
G = 2                        # head groups of 2 heads (128 chans) per core
NCORES = 8


def build_program():
    nc = bacc.Bacc("TRN2", target_bir_lowering=False, debug=False)

    # ---- DRAM I/O ----------------------------------------------------------
    srcT_d = nc.dram_tensor("srcT_in", [D, T], F32R, kind="ExternalInput")
    wq_d = nc.dram_tensor("wq", [D, KDC], F32R, kind="ExternalInput")
    wk_d = nc.dram_tensor("wk", [D, KDC], F32R, kind="ExternalInput")
    wv_d = nc.dram_tensor("wv", [D, VDC], F32R, kind="ExternalInput")
    wgate_d = nc.dram_tensor("wgate", [D, VDC], F32R, kind="ExternalInput")
    wg1_d = nc.dram_tensor("wg1", [D, 16], F32R, kind="ExternalInput")
    wg2b_d = nc.dram_tensor("wg2b", [17, KDC], F32R, kind="ExternalInput")
    wo_d = nc.dram_tensor("wo", [VDC, D], F32R, kind="ExternalInput")
    convdiag_d = nc.dram_tensor(
        "convdiag", [(2 * MIQ + MIV) * CONV, 128, 128], F32R, kind="ExternalInput"
    )
    maskc_d = nc.dram_tensor("maskc", [128, NCH], F32, kind="ExternalInput")
    out_d = nc.dram_tensor("out", [T, D], F32, kind="ExternalOutput")

    ident_np = np.eye(128, dtype=np.float32)
    u = np.triu(np.ones((128, 128), np.float32))  # U[s,t] = 1 iff s <= t
    ident_d = nc.inline_tensor(ident_np, "ident_c")
    triu2_d = nc.inline_tensor(np.concatenate([u, u], axis=1), "triu2_c")

    # ---- static SBUF -------------------------------------------------------
    srcT = nc.alloc_sbuf_tensor("srcT", [128, 8, T], F32R)      # src^T, d-major
    q_sb = nc.alloc_sbuf_tensor("q_sb", [128, MIQ, T], F32R)     # q then q~ (in place)
    k_sb = nc.alloc_sbuf_tensor("k_sb", [128, MIQ, T], F32R)     # k then k~
    v_sb = nc.alloc_sbuf_tensor("v_sb", [128, MIV, T], F32)
    gate_sb = nc.alloc_sbuf_tensor("gate_sb", [128, NCH, VDC], F32)  # silu(gate) → o*gate
    xgT = nc.alloc_sbuf_tensor("xgT", [17, T], F32R)            # (src@Wg1)^T + ones row
    spT = nc.alloc_sbuf_tensor("spT", [128, MIQ, T], F32)       # softplus(-gk_logit)
    bsum = nc.alloc_sbuf_tensor("bsum", [128, MIQ, T], F32)     # per-chunk cumsum of spT
    bCn = nc.alloc_sbuf_tensor("bCn", [128, MIQ, NCH], F32)     # -spsum_last/16 per chunk
    ssq_all = nc.alloc_sbuf_tensor("ssq_all", [128, NCH * 4], F32)   # col = c*4 + head
    rrms_all = nc.alloc_sbuf_tensor("rrms_all", [128, NCH * 4], F32)
    Eall = nc.alloc_sbuf_tensor("Eall", [128, MIQ, NCH], F32)   # exp(b_C) per chunk
    wo_sb = nc.alloc_sbuf_tensor("wo_sb", [128, MIV, D], F32R)
    wgate_sb = nc.alloc_sbuf_tensor("wgate_sb", [128, 8, VDC], F32R)
    wg1_sb = nc.alloc_sbuf_tensor("wg1_sb", [128, 8, 16], F32R)
    wg2b_sb = nc.alloc_sbuf_tensor("wg2b_sb", [17, KDC], F32R)
    maskc_sb = nc.alloc_sbuf_tensor("maskc_sb", [128, NCH], F32)
    ident = nc.alloc_sbuf_tensor("ident", [128, 128], F32)
    triu2 = nc.alloc_sbuf_tensor("triu2", [128, 256], F32)
    ones_sb = nc.alloc_sbuf_tensor("ones_sb", [128, 128], F32)
    Sblk = [nc.alloc_sbuf_tensor(f"Sblk{g}", [128, 256], F32R) for g in range(G)]
    qblk = [nc.alloc_sbuf_tensor(f"qblk{g}", [128, 256], F32R) for g in range(G)]
    negln8 = nc.alloc_sbuf_tensor("negln8", [128, 1], F32)
    eps_col = nc.alloc_sbuf_tensor("eps_col", [128, 1], F32)

    with tile.TileContext(nc) as tc:
        with (
            tc.tile_pool(name="scr", bufs=4) as scr,
            tc.tile_pool(name="ps_t", bufs=2, space="PSUM") as ps_t,
        ):
            # ---- phase 0: constants in (src streams first; the big late
            # weights go on the GpSimd DMA queue so they don't delay src) ----
            nc.sync.dma_start(out=ident[:], in_=ident_d[:])
            nc.sync.dma_start(out=triu2[:], in_=triu2_d[:])
            nc.sync.dma_start(out=maskc_sb[:], in_=maskc_d[:])
            nc.sync.dma_start(
                out=wg1_sb[:], in_=wg1_d[:].rearrange("(kt p) m -> p kt m", p=128)
            )
            nc.sync.dma_start(out=wg2b_sb[:], in_=wg2b_d[:])
            nc.vector.memset(ones_sb[:], 1.0)
            # row 16 must be ones (bias row); rows 0..15 are overwritten later
            nc.vector.memset(xgT[:].bitcast(F32), 1.0)
            nc.vector.memset(negln8[:], -LN8)
            nc.vector.memset(eps_col[:], EPS)
            for g in range(G):
                nc.vector.memset(Sblk[g][:].bitcast(F32), 0.0)
                nc.vector.memset(qblk[g][:].bitcast(F32), 0.0)

            # ---- phases 1+2: gk; projections+conv; gate --------------------
            # (src arrives pre-transposed from the host: one straight DMA)
            with (
                tc.tile_pool(name="wmi", bufs=4) as wmi_pool,
                tc.tile_pool(name="scr2", bufs=2) as scr2,
                tc.tile_pool(name="ps_proj", bufs=6, space="PSUM") as ps_proj,
            ):
                # eight separate DMAs, issued from all three DMA-capable
                # engines so the 4MB spreads across independent DMA paths
                dma_engs = [nc.sync, nc.scalar, nc.gpsimd]
                for kt in range(8):
                    dma_engs[kt % 3].dma_start(
                        out=srcT[:, kt, :],
                        in_=srcT_d[kt * 128:(kt + 1) * 128, :],
                    )

                # gk path first so the chunk loop's inputs are ready early:
                # xg^T = (src @ Wg1)^T with an appended ones row
                for nh in range(2):
                    p = ps_proj.tile([128, 512], F32, name="pp_xg", tag="pp")
                    for kt in range(8):
                        nc.tensor.matmul(
                            p[0:16, :],
                            wg1_sb[:, kt, :],
                            srcT[:, kt, nh * 512:(nh + 1) * 512],
                            start=(kt == 0),
                            stop=(kt == 7),
                        )
                    nc.vector.tensor_copy(
                        out=xgT[0:16, nh * 512:(nh + 1) * 512], in_=p[0:16, :]
                    )
                # spT = softplus(-(xg @ Wg2 + bg2)) = log(1 + exp(-logit))
                # (logits here are O(1), so exp(-x) cannot overflow).
                # All 4 Exps run before all 4 Lns so the ACT engine loads
                # each activation table once instead of thrashing.
                enxs = []
                for mi in range(MIQ):
                    for nh in range(2):
                        p = ps_proj.tile([128, 512], F32, name="pp_sp", tag="pp")
                        nc.tensor.matmul(
                            p[:],
                            wg2b_sb[:, mi * 128:(mi + 1) * 128],
                            xgT[:, nh * 512:(nh + 1) * 512],
                            start=True,
                            stop=True,
                        )
                        enx = scr2.tile(
                            [128, 512], F32, name="enx", tag="enx", bufs=4
                        )
                        nc.scalar.activation(enx[:], p[:], AF.Exp, scale=-1.0)
                        enxs.append((mi, nh, enx))
                for mi, nh, enx in enxs:
                    nc.scalar.activation(
                        spT[:, mi, nh * 512:(nh + 1) * 512], enx[:],
                        AF.Ln, bias=1.0,
                    )
                # per-chunk inclusive cumsum of spT, chunk-end columns
                for mi in range(MIQ):
                    for c in range(NCH):
                        csl = slice(c * 128, (c + 1) * 128)
                        nc.vector.tensor_tensor_scan(
                            out=bsum[:, mi, csl],
                            data0=ones_sb[:],
                            data1=spT[:, mi, csl],
                            initial=0.0,
                            op0=OP.mult,
                            op1=OP.add,
                        )
                        nc.vector.tensor_scalar_mul(
                            bCn[:, mi, c:c + 1],
                            bsum[:, mi, c * 128 + 127:c * 128 + 128],
                            -1.0 / GATE_NORM,
                        )
                    nc.scalar.activation(Eall[:, mi, :], bCn[:, mi, :], AF.Exp)

                def conv_proj(w_dram, diag_base, dst, mi_count, prio_tag):
                    """dst[:, mi, :] = silu(conv4(src @ W[:, mi-block]))^T.

                    The pre-activation is staged (padded) in SBUF via ACT
                    copies; the conv itself runs on the PE as 4 shifted
                    diagonal-matrix matmuls accumulating in PSUM (the DVE was
                    the kernel-wide bottleneck, the PE has slack here).
                    """
                    for mi in range(mi_count):
                        w_mi = wmi_pool.tile(
                            [128, 8, 128], F32R, name="w_mi", tag="w_mi"
                        )
                        nc.sync.dma_start(
                            out=w_mi[:],
                            in_=w_dram[:, mi * 128:(mi + 1) * 128].rearrange(
                                "(kt p) m -> p kt m", p=128
                            ),
                        )
                        dg = wmi_pool.tile([128, 4, 128], F32R, name="dg", tag="dg")
                        nc.sync.dma_start(
                            out=dg[:],
                            in_=convdiag_d[4 * (diag_base + mi):
                                           4 * (diag_base + mi) + 4].rearrange(
                                "j p m -> p j m"
                            ),
                        )
                        pre = scr2.tile([128, 1027], F32R, name="pre", tag="pre")
                        nc.gpsimd.memset(pre[:, 0:3].bitcast(F32), 0.0)
                        for nh in range(2):
                            p = ps_proj.tile(
                                [128, 512], F32, name=f"pp_{prio_tag}", tag="pp"
                            )
                            for kt in range(8):
                                nc.tensor.matmul(
                                    p[:],
                                    w_mi[:, kt, :],
                                    srcT[:, kt, nh * 512:(nh + 1) * 512],
                                    start=(kt == 0),
                                    stop=(kt == 7),
                                )
                            nc.scalar.copy(
                                out=pre[:, 3 + nh * 512:3 + (nh + 1) * 512], in_=p[:]
                            )
                        # causal conv: out[t] = sum_j pre_padded[t + j] w[:, j]
                        # as 4 diag(w_j) matmuls with shifted moving operands
                        dst_mi = dst[:, mi, :]
                        for nh in range(2):
                            cp = ps_proj.tile(
                                [128, 512], F32, name=f"cp_{prio_tag}", tag="pp"
                            )
                            for j in range(CONV):
                                nc.tensor.matmul(
                                    cp[:],
                                    dg[:, j, :],
                                    pre[:, nh * 512 + j:nh * 512 + j + 512],
                                    start=(j == 0),
                                    stop=(j == 3),
                                )
                            # silu(x) = x * sigmoid(x), evacuating the PSUM
                            sg = scr2.tile([128, 512], F32, name="sg", tag="sg")
                            nc.scalar.activation(sg[:], cp[:], AF.Sigmoid)
                            nc.vector.tensor_mul(
                                dst_mi[:, nh * 512:(nh + 1) * 512], cp[:], sg[:]
                            )

                nc.gpsimd.dma_start(
                    out=wgate_sb[:],
                    in_=wgate_d[:].rearrange("(kt p) m -> p kt m", p=128),
                )
                conv_proj(wq_d[:], 0, q_sb, MIQ, "q")
                conv_proj(wk_d[:], MIQ, k_sb, MIQ, "k")

                # q~ = q exp(b)/8 and k~ = k exp(-b), full-T, in place, with
                # f32r rounding on the write (they feed matmuls from here on).
                # Emitted right after the q/k convs so the chunk loop can start
                # while the v conv and gate projection are still running.
                for mi in range(MIQ):
                    texp = scr2.tile([128, 1024], F32, name="texp", tag="texp", bufs=1)
                    nc.scalar.activation(
                        texp[:], bsum[:, mi, :], AF.Exp,
                        scale=-1.0 / GATE_NORM, bias=negln8[:],
                    )
                    for half in range(2):
                        hsl = slice(half * 512, (half + 1) * 512)
                        nc.vector.tensor_mul(
                            q_sb[:, mi, hsl], q_sb[:, mi, hsl], texp[:, hsl]
                        )
                    texp2 = scr2.tile([128, 1024], F32, name="texp2", tag="texp", bufs=1)
                    nc.scalar.activation(
                        texp2[:], bsum[:, mi, :], AF.Exp, scale=1.0 / GATE_NORM,
                    )
                    for half in range(2):
                        hsl = slice(half * 512, (half + 1) * 512)
                        nc.vector.tensor_mul(
                            k_sb[:, mi, hsl], k_sb[:, mi, hsl], texp2[:, hsl]
                        )

                # gate: silu(src @ Wgate), natural (t-major) layout — dense PE
                # work that overlaps the conv's DVE stretch
                for mt in range(8):
                    p = ps_proj.tile([128, 512], F32, name="pp_gate", tag="pp")
                    for kt in range(8):
                        nc.tensor.matmul(
                            p[:],
                            srcT[:, kt, mt * 128:(mt + 1) * 128],
                            wgate_sb[:, kt, :],
                            start=(kt == 0),
                            stop=(kt == 7),
                        )
                    sgg = scr2.tile([128, 512], F32, name="sgg", tag="sgg")
                    nc.scalar.activation(sgg[:], p[:], AF.Sigmoid)
                    nc.vector.tensor_mul(gate_sb[:, mt, :], p[:], sgg[:])

                nc.gpsimd.dma_start(
                    out=wo_sb[:], in_=wo_d[:].rearrange("(h p) m -> p h m", p=128)
                )
                conv_proj(wv_d[:], 2 * MIQ, v_sb, MIV, "v")

            # ---- phase 4: GLA chunk recurrence -----------------------------
            with (
                tc.tile_pool(name="ogT_pool", bufs=1) as ogT_pool,
            ):
                ogT = ogT_pool.tile([128, MIV, T], F32R, name="ogT")
                with (
                    tc.tile_pool(name="ps_h", bufs=4, space="PSUM") as ps_h,
                    tc.tile_pool(name="ps_o", bufs=2, space="PSUM") as ps_o_pool,
                ):
                    for c in range(NCH):
                        csl = slice(c * 128, (c + 1) * 128)
                        for g in range(G):
                            qt = q_sb[:, g, csl]
                            kt_ = k_sb[:, g, csl]
                            e_col = Eall[:, g, c:c + 1]
                            # k^ = k~ * exp(b_C)  (per-partition scalar)
                            kh_s = scr.tile([128, 128], F32, name="kh_s", tag="kh_s")
                            nc.vector.tensor_scalar_mul(kh_s[:], k_sb[:, g, csl], e_col)
                            nc.vector.tensor_copy(
                                out=qblk[g][0:64, 0:128], in_=qt[0:64, :]
                            )
                            nc.vector.tensor_copy(
                                out=qblk[g][64:128, 128:256], in_=qt[64:128, :]
                            )
                            # A~[s, t] for both heads: (s, [t_h0 | t_h1])
                            ps_a = ps_h.tile([128, 256], F32, name="ps_a", tag="ps_h")
                            nc.tensor.matmul(
                                ps_a[:], kt_, qblk[g][:], start=True, stop=True
                            )
                            a_sb = scr.tile([128, 256], F32R, name="a_sb", tag="a_sb")
                            nc.vector.tensor_mul(a_sb[:], ps_a[:], triu2[:])
                            # v chunk, time-major (+ padding mask)
                            ps_v = ps_h.tile([128, 256], F32, name="ps_v", tag="ps_h")
                            nc.tensor.matmul(
                                ps_v[:, 0:128], v_sb[:, 2 * g, csl], ident[:],
                                is_transpose=True, start=True, stop=False,
                                skip_group_check=True,
                            )
                            nc.tensor.matmul(
                                ps_v[:, 128:256], v_sb[:, 2 * g + 1, csl], ident[:],
                                is_transpose=True, start=False, stop=True,
                                skip_group_check=True,
                            )
                            vnat = scr.tile([128, 256], F32R, name="vnat", tag="vnat")
                            nc.vector.tensor_scalar_mul(
                                vnat[:], ps_v[:], maskc_sb[:, c:c + 1]
                            )
                            # k^ chunk, time-major
                            ps_k = ps_t.tile([128, 256], F32, name="ps_k", tag="pst")
                            nc.tensor.transpose(ps_k[:, 0:128], kh_s[:], ident[:])
                            khnat = scr.tile([128, 128], F32R, name="khnat", tag="khnat")
                            nc.scalar.copy(out=khnat[:], in_=ps_k[:, 0:128])
                            # o = A~^T v (intra) + q~ @ S (inter)
                            ps_o = ps_o_pool.tile([128, 256], F32, name="ps_o", tag="ps_o")
                            nc.tensor.matmul(
                                ps_o[:, 0:128], a_sb[:, 0:128], vnat[:, 0:128],
                                start=True, stop=False, skip_group_check=True,
                            )
                            nc.tensor.matmul(
                                ps_o[:, 128:256], a_sb[:, 128:256], vnat[:, 128:256],
                                start=False, stop=False, skip_group_check=True,
                            )
                            nc.tensor.matmul(
                                ps_o[:], qt, Sblk[g][:],
                                start=False, stop=True, skip_group_check=True,
                            )
                            # state update: S = diag(exp(b_C)) S + k^T v
                            ps_s = ps_h.tile([128, 256], F32, name="ps_s", tag="ps_h")
                            nc.tensor.matmul(
                                ps_s[:], khnat[:], vnat[:], start=True, stop=True
                            )
                            nc.vector.scalar_tensor_tensor(
                                out=Sblk[g][0:64, 0:128],
                                in0=Sblk[g][0:64, 0:128],
                                scalar=e_col[0:64, :],
                                in1=ps_s[0:64, 0:128],
                                op0=OP.mult,
                                op1=OP.add,
                            )
                            nc.vector.scalar_tensor_tensor(
                                out=Sblk[g][64:128, 128:256],
                                in0=Sblk[g][64:128, 128:256],
                                scalar=e_col[64:128, :],
                                in1=ps_s[64:128, 128:256],
                                op0=OP.mult,
                                op1=OP.add,
                            )
                            # evacuate o: multiply the swish gate in-place into
                            # gate_sb, and collect per-head sums of squares
                            for lh in range(2):
                                sqd = scr.tile([128, 128], F32, name="sqd", tag="sqd")
                                idx = c * 4 + 2 * g + lh
                                nc.scalar.activation(
                                    sqd[:], ps_o[:, lh * 128:(lh + 1) * 128],
                                    AF.Square,
                                    accum_out=ssq_all[:, idx:idx + 1],
                                )
                            gsl = slice(g * 256, (g + 1) * 256)
                            nc.vector.tensor_mul(
                                gate_sb[:, c, gsl], ps_o[:], gate_sb[:, c, gsl]
                            )

                # ---- tail: rrms, gate-scale, transpose, and the output
                # projection all pipelined per chunk (chunk c is exactly
                # output row-tile mt=c, so each chunk streams straight
                # through Wo and out to DRAM)
                lnr = scr.tile([128, NCH * 4], F32, name="lnr", tag="lnr")
                nc.scalar.activation(
                    lnr[:], ssq_all[:], AF.Ln, scale=1.0 / DV, bias=eps_col[:]
                )
                nc.scalar.activation(rrms_all[:], lnr[:], AF.Exp, scale=-0.5)
                with (
                    tc.tile_pool(name="ps_out", bufs=3, space="PSUM") as ps_out,
                    tc.tile_pool(name="stage", bufs=3) as stage_pool,
                ):
                    for c in range(NCH):
                        csl = slice(c * 128, (c + 1) * 128)
                        rr = rrms_all[:, c * 4:(c + 1) * 4, None].to_broadcast(
                            (128, 4, 128)
                        )
                        nc.vector.tensor_mul(
                            gate_sb[:, c, :].rearrange("p (h x) -> p h x", h=4),
                            gate_sb[:, c, :].rearrange("p (h x) -> p h x", h=4),
                            rr,
                        )
                        for h in range(0, 4, 2):
                            ps_g = ps_t.tile([128, 256], F32, name="ps_g", tag="pst")
                            nc.tensor.matmul(
                                ps_g[:, 0:128], gate_sb[:, c, h * 128:(h + 1) * 128],
                                ident[:], is_transpose=True, start=True, stop=False,
                                skip_group_check=True,
                            )
                            nc.tensor.matmul(
                                ps_g[:, 128:256],
                                gate_sb[:, c, (h + 1) * 128:(h + 2) * 128],
                                ident[:], is_transpose=True, start=False, stop=True,
                                skip_group_check=True,
                            )
                            nc.scalar.copy(
                                out=ogT[:, h:h + 2, csl],
                                in_=ps_g[:].rearrange("p (a b) -> p a b", a=2),
                            )
                        for nh in range(2):
                            p = ps_out.tile([128, 512], F32, name="p_out", tag="p_out")
                            for h in range(4):
                                nc.tensor.matmul(
                                    p[:],
                                    ogT[:, h, csl],
                                    wo_sb[:, h, nh * 512:(nh + 1) * 512],
                                    start=(h == 0),
                                    stop=(h == 3),
                                )
                            stage = stage_pool.tile(
                                [128, 512], F32, name="stage", tag="stage"
                            )
                            if (c + nh) % 2 == 0:
                                nc.vector.tensor_copy(out=stage[:], in_=p[:])
                            else:
                                nc.scalar.copy(out=stage[:], in_=p[:])
                            nc.gpsimd.dma_start(
                                out=out_d[c * 128:(c + 1) * 128,
                                          nh * 512:(nh + 1) * 512],
                                in_=stage[:],
                            )

    nc.compile()
    return nc


_NC_CACHE = None


def _get_program():
    global _NC_CACHE
    if _NC_CACHE is None:
        _NC_CACHE = build_program()
    return _NC_CACHE


def shard_inputs(
    src, valid_mask, Wq, Wk, Wv, conv_q_w, conv_k_w, conv_v_w,
    Wg1, Wg2, bg2, Wgate, rms_w, Wo,
):
    """Build the 8 per-core input maps."""
    f = np.float32
    src = np.asarray(src, f)
    valid_mask = np.asarray(valid_mask)
    in_maps = []
    wo_scaled = np.asarray(Wo, f) * np.tile(np.asarray(rms_w, f), VD // DV)[:, None]
    for core in range(NCORES):
        b, hg = core // 2, core % 2
        qs = slice(hg * KDC, (hg + 1) * KDC)
        vs = slice(hg * VDC, (hg + 1) * VDC)
        wg2b = np.concatenate(
            [np.asarray(Wg2, f)[:, qs], np.asarray(bg2, f)[None, qs]], axis=0
        )

        # one (128,128) diagonal matrix per (channel-tile, tap) for the
        # PE-side depthwise conv: q tiles, k tiles, then v tiles
        conv_diag = np.zeros(((2 * MIQ + MIV) * CONV, 128, 128), f)
        tiles = []
        for w, sel, n in ((conv_q_w, qs, MIQ), (conv_k_w, qs, MIQ),
                          (conv_v_w, vs, MIV)):
            wa = np.asarray(w, f)[sel]
            tiles.extend(wa[i * 128:(i + 1) * 128] for i in range(n))
        for ti, wt in enumerate(tiles):      # wt: (128, 4)
            for j in range(CONV):
                np.fill_diagonal(conv_diag[ti * CONV + j], wt[:, j])

        in_maps.append({
            "srcT_in": np.ascontiguousarray(src[b].T),
            "wq": np.ascontiguousarray(np.asarray(Wq, f)[:, qs]),
            "wk": np.ascontiguousarray(np.asarray(Wk, f)[:, qs]),
            "wv": np.ascontiguousarray(np.asarray(Wv, f)[:, vs]),
            "wgate": np.ascontiguousarray(np.asarray(Wgate, f)[:, vs]),
            "wg1": np.ascontiguousarray(np.asarray(Wg1, f)),
            "wg2b": np.ascontiguousarray(wg2b),
            "wo": np.ascontiguousarray(wo_scaled[vs, :]),
            "convdiag": conv_diag,
            "maskc": np.ascontiguousarray(
                valid_mask[b].astype(f).reshape(NCH, 128).T
            ),
        })
    return in_maps


def kernel(**inputs):
    nc = _get_program()
    in_maps = shard_inputs(**inputs)
    res = run_bass_kernel_spmd(nc, in_maps, list(range(NCORES)))
    out = np.zeros((B, T, D), np.float32)
    for core in range(NCORES):
        out[core // 2] += res.results[core]["out"]
    return out


if __name__ == "__main__":
    prog = _get_program()
    print("program built OK")
